# revision 26
# baseline (speedup 1.0000x reference)
"""GPT-2 (L=12, E=1024, H=16, T=1024, B=8) forward on 8 Trainium2 NeuronCores.

Strategy: data-parallel over batch (1 sequence per core) for the 12 transformer
layers; vocab-parallel lm_head (each core computes logits for a V/8 shard of the
vocabulary for all 8 sequences) as a second tiny NEFF, with the 8 last-position
hidden vectors gathered on host between the phases.

v2 (all-bf16 matmul datapath, fp32 residual/PSUM):
  - weights pre-cast to bf16 and pre-tiled on host so every weight load is one
    large contiguous DMA
  - LN output transposed via DMA-transpose (xbar) instead of PE transposes
  - attention: heads processed in pairs (even head on partitions 0-63, odd on
    64-127) so the two QK matmuls row-tile the PE array concurrently
  - softmax denominator comes free from an appended ones-column in V
    (out row 64 of the AV matmul), replacing the ones-matmul accumulation
  - causal mask applied as a post-exp zeroing affine_select on GpSimd
  - layer 11 computes Q/attention/proj/MLP only for the last 128 tokens
    (only the last position's logits are needed downstream)

Host-side preprocessing (all linear folds, no model compute):
  - embedding gather x0 = wte[idx] + wpe  (pure indexing)
  - layernorm scale folded into the following matmul weights
  - 1/sqrt(D) folded into W_q
  - wte transposed (+ lnf scale) for the lm_head
"""

import os
import sys

import numpy as np

sys.path.insert(0, "/opt/trn_rl_repo")

V, BLK, L, H, E = 50257, 1024, 12, 16, 1024
T = 1024
D = E // H  # 64
NCORES = 8
E3 = 3 * E
E4 = 4 * E
NTT = T // 128  # 8 token tiles
NEO = E // 128  # 8 embed tiles
VSH = (V + NCORES - 1) // NCORES  # 6283 vocab shard
VSP = 13 * 512  # 6656 padded shard width
NL = int(os.environ.get("GPT_NL", str(L)))
TRUNC_LAST = os.environ.get("GPT_TRUNC", "1") == "1"

_CACHE = {}

# attT column offsets for the compacted causal layout, per q-window start
def _offs(qlo):
    offs, col = [], 0
    for j in range(NTT):
        offs.append(col)
        col += T - max(j * 128, qlo)
    return offs, col


def _build_phase1(nl):
    import concourse.mybir as mybir
    import concourse.tile as tile
    from concourse import bacc

    f32 = mybir.dt.float32
    bf16 = mybir.dt.bfloat16
    AF = mybir.ActivationFunctionType
    ALU = mybir.AluOpType

    nc = bacc.Bacc("TRN2", target_bir_lowering=False)

    x0 = nc.dram_tensor("x0", [T, E], f32, kind="ExternalInput")
    wqk = nc.dram_tensor("wqk", [nl, 16, 128, NEO, 128], bf16, kind="ExternalInput")
    wv = nc.dram_tensor("wv", [nl, 2, 128, NEO, 512], bf16, kind="ExternalInput")
    wproj = nc.dram_tensor("wproj", [nl, 2, 128, NEO, 512], bf16, kind="ExternalInput")
    wfc = nc.dram_tensor("wfc", [nl, 32, 128, NEO, 128], bf16, kind="ExternalInput")
    wfc2 = nc.dram_tensor("wfc2", [nl, 4, 2, 128, NEO, 512], bf16, kind="ExternalInput")
    xlast = nc.dram_tensor("xlast", [1, E], f32, kind="ExternalOutput")

    ATT_W = 4608  # full compacted attT width per head
    VGW = 16 * 65  # V block per tt: 16 heads x (64 cols + ones col)

    with tile.TileContext(nc) as tc:
        import contextlib

        ctx = contextlib.ExitStack()
        with ctx:
            singles = ctx.enter_context(tc.tile_pool(name="singles", bufs=1))
            avs = ctx.enter_context(tc.tile_pool(name="avs", bufs=2))
            wl = ctx.enter_context(tc.tile_pool(name="wl", bufs=4))  # lhsT ct tiles
            wr = ctx.enter_context(tc.tile_pool(name="wr", bufs=3))  # rhs tiles
            hpool = ctx.enter_context(tc.tile_pool(name="hpool", bufs=2))
            stat = ctx.enter_context(tc.tile_pool(name="stat", bufs=2))
            bc = ctx.enter_context(tc.tile_pool(name="bc", bufs=2))
            dram = ctx.enter_context(tc.tile_pool(name="dram", bufs=2, space="DRAM"))
            scrA = ctx.enter_context(tc.tile_pool(name="scrA", bufs=2))
            scrB = ctx.enter_context(tc.tile_pool(name="scrB", bufs=1))
            # PSUM: pmm (evacuated mm outputs) and pqk (attention scores) are
            # separate pools so Q/K/V/MLP matmuls never block on the exp chain
            pmm = ctx.enter_context(tc.tile_pool(name="pmm", bufs=3, space="PSUM"))
            pqk = ctx.enter_context(tc.tile_pool(name="pqk", bufs=1, space="PSUM"))
            pav = ctx.enter_context(tc.tile_pool(name="pav", bufs=1, space="PSUM"))

            # ---- persistent tiles ----
            X = singles.tile([128, NTT, T], f32)  # residual [tp, tt, e]
            HT = singles.tile([128, NEO, T], bf16)  # ln-out transposed [ep, eo, t]
            AOT = singles.tile([128, NEO, T], bf16)  # attn outT [cp, co, t]
            eps_t = singles.tile([128, 1], f32)
            nc.gpsimd.memset(eps_t, 1e-5)

            # ---- load x0 ----
            x0v = x0[:, :].rearrange("(tt p) e -> p tt e", p=128)
            for tt in range(NTT):
                nc.sync.dma_start(X[:, tt, :], x0v[:, tt, :])

            def layernorm(tts, l, which):
                """LN(X[:,tt,:]) -> HT[:, :, tt*128:(tt+1)*128] via DMA transpose."""
                for tt in tts:
                    st = stat.tile([128, 2, 6], f32, tag="bnst", name=f"st{l}_{which}_{tt}")
                    for c in range(2):
                        nc.vector.bn_stats(st[:, c, :], X[:, tt, c * 512 : (c + 1) * 512])
                    mv = stat.tile([128, 2], f32, tag="bnmv", name=f"mv{l}_{which}_{tt}")
                    nc.vector.bn_aggr(mv, st)
                    rstd = stat.tile([128, 1], f32, tag="rstd", name=f"rs{l}_{which}_{tt}")
                    nc.scalar.activation(rstd, mv[:, 1:2], AF.Sqrt, bias=eps_t)
                    nc.vector.reciprocal(rstd, rstd)
                    h = hpool.tile([128, T], bf16, tag="h", name=f"h{l}_{which}_{tt}")
                    nc.vector.tensor_scalar(
                        out=h,
                        in0=X[:, tt, :],
                        scalar1=mv[:, 0:1],
                        scalar2=rstd,
                        op0=ALU.subtract,
                        op1=ALU.mult,
                    )
                    nc.scalar.dma_start_transpose(HT[:, :, tt * 128 : (tt + 1) * 128], h)

            def mm_lhsw(dst_fn, wdram_ct, t_lo, t_hi, act, nm):
                """dst(ch0, w) <- W_ct.T @ HT[:, :, ch0:ch0+w] per 512 chunk."""
                wt = wl.tile([128, NEO, 128], bf16, tag="wl", name=f"wt{nm}")
                nc.sync.dma_start(wt, wdram_ct)
                for ch0 in range(t_lo, t_hi, 512):
                    w = min(512, t_hi - ch0)
                    pt = pmm.tile([128, 512], f32, tag="pmm", name=f"pt{nm}_{ch0}")
                    for eo in range(NEO):
                        nc.tensor.matmul(
                            pt[:, :w],
                            wt[:, eo, :],
                            HT[:, eo, ch0 : ch0 + w],
                            start=(eo == 0),
                            stop=(eo == NEO - 1),
                        )
                    if act is None:
                        nc.vector.tensor_copy(dst_fn(ch0, w), pt[:, :w])
                    else:
                        nc.scalar.activation(dst_fn(ch0, w), pt[:, :w], act)

            # LN1 of layer 0; later layers' LN1 is emitted inside the previous
            # layer's fc2 tail so it pipelines with the remaining MLP matmuls
            layernorm(range(NTT), 0, 0)

            for l in range(nl):
                last = TRUNC_LAST and (l == nl - 1) and (nl == L)
                qlo = T - 128 if last else 0
                tts = [NTT - 1] if last else list(range(NTT))
                offs, attw = _offs(qlo)

                # ===== attention =====
                VA = scrB.tile([128, NTT * VGW + 2 * ATT_W], bf16, tag="scrB", name=f"va{l}")
                VG = VA[:, : NTT * VGW].rearrange("p (tt h c) -> p tt h c", tt=NTT, h=16)

                def attT(slot, col, n):  # slot = head_in_pair (one pair in flight)
                    return VA[:, NTT * VGW + slot * ATT_W + col : NTT * VGW + slot * ATT_W + col + n]

                # ones columns for the free softmax denominator
                nc.gpsimd.memset(VG[:, :, :, 64:65], 1.0)

                # V for both groups first (so AV can chase QK/exp per pair)
                for g in range(2):
                    wvt = wr.tile([128, NEO, 512], bf16, tag="wr", name=f"wv{l}_{g}")
                    nc.sync.dma_start(wvt, wv[l, g])
                    for tt in range(NTT):
                        pv = pmm.tile([128, 512], f32, tag="pmm", name=f"pv{l}_{g}_{tt}")
                        for eo in range(NEO):
                            nc.tensor.matmul(
                                pv,
                                HT[:, eo, tt * 128 : (tt + 1) * 128],
                                wvt[:, eo, :],
                                start=(eo == 0),
                                stop=(eo == NEO - 1),
                            )
                        nc.scalar.activation(
                            VG[:, tt, g * 8 : (g + 1) * 8, 0:64],
                            pv.rearrange("p (h c) -> p h c", h=8),
                            AF.Copy,
                        )

                QT = scrA.tile([128, NEO, T], bf16, tag="scrA", name=f"qt{l}")
                KT = scrA.tile([128, NEO, T], bf16, tag="scrA", name=f"kt{l}")

                for p in range(NEO):  # head pair p = heads (2p, 2p+1)
                    mm_lhsw(lambda c0, w, p=p: QT[:, p, c0 : c0 + w], wqk[l, p], qlo, T, None, f"q{l}_{p}")
                    mm_lhsw(lambda c0, w, p=p: KT[:, p, c0 : c0 + w], wqk[l, 8 + p], 0, T, None, f"k{l}_{p}")

                    # --- QK for the pair (row-tiled: even rows 0-63, odd 64-127) ---
                    # consecutive k-blocks share a psum tile (and one exp) while
                    # their combined q-width fits the 2-bank tile
                    jgroups, cur, curw = [], [], 0
                    for j in range(NTT):
                        qn = T - max(j * 128, qlo)
                        if curw + qn > 1024:
                            jgroups.append(cur)
                            cur, curw = [], 0
                        cur.append((j, curw, qn))
                        curw += qn
                    jgroups.append(cur)
                    for gi, grp in enumerate(jgroups):
                        gw = sum(qn for _, _, qn in grp)
                        pq_t = pqk.tile([128, 2 * T], f32, tag="pqk", name=f"pq{l}_{p}_{gi}")
                        pe_t = pq_t[:, 0:T]
                        po_t = pq_t[:, T : 2 * T]
                        for j, lo, qn in grp:
                            qs = max(j * 128, qlo)
                            for ch0 in range(0, qn, 512):
                                w = min(512, qn - ch0)
                                nc.tensor.matmul(
                                    pe_t[:, lo + ch0 : lo + ch0 + w],
                                    KT[0:64, p, j * 128 : (j + 1) * 128],
                                    QT[0:64, p, qs + ch0 : qs + ch0 + w],
                                    start=True,
                                    stop=True,
                                )
                                nc.tensor.matmul(
                                    po_t[:, lo + ch0 : lo + ch0 + w],
                                    KT[64:128, p, j * 128 : (j + 1) * 128],
                                    QT[64:128, p, qs + ch0 : qs + ch0 + w],
                                    start=True,
                                    stop=True,
                                )
                        o0 = offs[grp[0][0]]
                        nc.scalar.activation(attT(0, o0, gw), pe_t[:, :gw], AF.Exp)
                        nc.scalar.activation(attT(1, o0, gw), po_t[:, :gw], AF.Exp)
                        for j, lo, qn in grp:
                            if j * 128 >= qlo:  # diagonal: zero strict upper triangle
                                for hh in range(2):
                                    nc.gpsimd.affine_select(
                                        out=attT(hh, offs[j], 128),
                                        in_=attT(hh, offs[j], 128),
                                        compare_op=ALU.is_ge,
                                        fill=0.0,
                                        base=0,
                                        pattern=[[1, 128]],
                                        channel_multiplier=-1,
                                    )

                    # --- AV + free denominator for both heads of the pair ---
                    for hh in range(2):
                        h_glob = 2 * p + hh
                        wd = T - qlo
                        av_s = avs.tile([65, 1024], f32, tag="avs", name=f"avs{l}_{h_glob}")
                        for ca0 in range(qlo, T, 512):
                            cw = min(512, T - ca0)
                            avp = pav.tile([65, 512], f32, tag="pav", name=f"av{l}_{h_glob}_{ca0}")
                            js = [j for j in range(NTT) if j * 128 < ca0 + cw]
                            for ji, j in enumerate(js):
                                s = max(ca0, j * 128)
                                w = ca0 + cw - s
                                nc.tensor.matmul(
                                    avp[:, s - ca0 : s - ca0 + w],
                                    VG[:, j, h_glob, :],
                                    attT(hh, offs[j] + s - max(j * 128, qlo), w),
                                    start=(ji == 0),
                                    stop=(ji == len(js) - 1),
                                    skip_group_check=True,
                                )
                            # evacuate value rows + denominator row per chunk,
                            # freeing the 1-bank PSUM accumulator quickly
                            nc.vector.tensor_copy(
                                av_s[:, ca0 - qlo : ca0 - qlo + cw], avp[:, :cw]
                            )
                        # full-tile AP (partition 0, 65 rows): reciprocal_approx_fast
                        # rejects single-partition slices at nonzero base partition
                        rden = stat.tile([65, 1024], f32, tag="rden", name=f"rd{l}_{h_glob}")
                        nc.vector.reciprocal_approx_fast(out=rden[:, :wd], in_=av_s[:, :wd])
                        rdd = dram.tile([1, 1024], f32, tag="rdd", name=f"rdd{l}_{h_glob}")
                        nc.scalar.dma_start(rdd[:, :wd], rden[64:65, :wd])
                        rdb = bc.tile([64, 1024], f32, tag="rdb", name=f"rdb{l}_{h_glob}")
                        nc.scalar.dma_start(rdb[:, :wd], rdd[:, :wd].to_broadcast([64, wd]))
                        co, ro2 = h_glob // 2, (h_glob % 2) * 64
                        nc.vector.tensor_tensor(
                            AOT[ro2 : ro2 + 64, co, qlo:T], av_s[0:64, :wd], rdb[:, :wd], ALU.mult
                        )

                # ===== proj + residual =====
                wp0 = wr.tile([128, NEO, 512], bf16, tag="wr", name=f"wp0{l}")
                wp1 = wr.tile([128, NEO, 512], bf16, tag="wr", name=f"wp1{l}")
                nc.sync.dma_start(wp0, wproj[l, 0])
                nc.sync.dma_start(wp1, wproj[l, 1])
                for tt in tts:
                    for chi, wpt in ((0, wp0), (1, wp1)):
                        pp = pmm.tile([128, 512], f32, tag="pmm", name=f"pp{l}_{tt}_{chi}")
                        for k in range(NEO):
                            nc.tensor.matmul(
                                pp,
                                AOT[:, k, tt * 128 : (tt + 1) * 128],
                                wpt[:, k, :],
                                start=(k == 0),
                                stop=(k == NEO - 1),
                            )
                        xs = X[:, tt, chi * 512 : chi * 512 + 512]
                        nc.vector.tensor_tensor(xs, xs, pp, ALU.add)
                    # LN2 for this token tile rides the proj loop so fc1 never
                    # waits on a serialized LN chain after proj completes
                    layernorm([tt], l, 1)

                # ===== mlp =====  (LN2 already emitted inside the proj loop)
                FC2A = scrB.tile([128, NTT, T], f32, tag="scrB", name=f"fc2a{l}")
                for slab in range(4):
                    H1T = scrA.tile([128, NEO, T], bf16, tag="scrA", name=f"h1t{l}_{slab}")
                    for ct in range(NEO):
                        mm_lhsw(
                            lambda c0, w, ct=ct, H1T=H1T: H1T[:, ct, c0 : c0 + w],
                            wfc[l, slab * 8 + ct],
                            qlo,
                            T,
                            AF.Gelu_apprx_tanh,
                            f"f{l}_{slab}_{ct}",
                        )
                    w20 = wr.tile([128, NEO, 512], bf16, tag="wr", name=f"w20{l}_{slab}")
                    w21 = wr.tile([128, NEO, 512], bf16, tag="wr", name=f"w21{l}_{slab}")
                    nc.sync.dma_start(w20, wfc2[l, slab, 0])
                    nc.sync.dma_start(w21, wfc2[l, slab, 1])
                    for tt in tts:
                        for chi, w2t in ((0, w20), (1, w21)):
                            p2 = pmm.tile([128, 512], f32, tag="pmm", name=f"p2{l}_{slab}_{tt}_{chi}")
                            for k in range(NEO):
                                nc.tensor.matmul(
                                    p2,
                                    H1T[:, k, tt * 128 : (tt + 1) * 128],
                                    w2t[:, k, :],
                                    start=(k == 0),
                                    stop=(k == NEO - 1),
                                )
                            sl = slice(chi * 512, chi * 512 + 512)
                            if slab == 0:
                                nc.vector.tensor_tensor(FC2A[:, tt, sl], X[:, tt, sl], p2, ALU.add)
                            elif slab < 3:
                                nc.vector.tensor_tensor(FC2A[:, tt, sl], FC2A[:, tt, sl], p2, ALU.add)
                            else:
                                nc.vector.tensor_tensor(X[:, tt, sl], FC2A[:, tt, sl], p2, ALU.add)
                        if slab == 3 and l + 1 < nl:
                            # next layer's LN1 for this token tile rides the
                            # fc2 tail so the layer boundary never idles PE
                            layernorm([tt], l + 1, 0)

            # ===== final layernorm on last token tile, emit last row =====
            st = stat.tile([128, 2, 6], f32, tag="bnst", name="stf")
            for c in range(2):
                nc.vector.bn_stats(st[:, c, :], X[:, NTT - 1, c * 512 : (c + 1) * 512])
            mv = stat.tile([128, 2], f32, tag="bnmv", name="mvf")
            nc.vector.bn_aggr(mv, st)
            rstd = stat.tile([128, 1], f32, tag="rstd", name="rsf")
            nc.scalar.activation(rstd, mv[:, 1:2], AF.Sqrt, bias=eps_t)
            nc.vector.reciprocal(rstd, rstd)
            xn = hpool.tile([128, T], f32, tag="xn", name="xnf", bufs=1)
            nc.vector.tensor_scalar(
                out=xn,
                in0=X[:, NTT - 1, :],
                scalar1=mv[:, 0:1],
                scalar2=rstd,
                op0=ALU.subtract,
                op1=ALU.mult,
            )
            nc.sync.dma_start(xlast[:, :], xn[127:128, :])

    nc.compile()
    return nc


def _build_phase2():
    import concourse.mybir as mybir
    import concourse.tile as tile
    from concourse import bacc

    f32 = mybir.dt.float32
    bf16 = mybir.dt.bfloat16
    AF = mybir.ActivationFunctionType

    nc = bacc.Bacc("TRN2", target_bir_lowering=False)
    xt_d = nc.dram_tensor("xt", [128, NEO, NCORES], bf16, kind="ExternalInput")
    wtet = nc.dram_tensor("wtet", [VSP // 512, 128, NEO, 512], bf16, kind="ExternalInput")
    lg = nc.dram_tensor("lg", [NCORES, VSP], f32, kind="ExternalOutput")

    with tile.TileContext(nc) as tc:
        with (
            tc.tile_pool(name="s", bufs=1) as s,
            tc.tile_pool(name="w", bufs=4) as w,
            tc.tile_pool(name="o", bufs=4) as o,
            tc.tile_pool(name="p", bufs=4, space="PSUM") as p,
        ):
            xt = s.tile([128, NEO, NCORES], bf16)
            nc.sync.dma_start(xt, xt_d[:, :, :])
            for vc in range(VSP // 512):
                wt = w.tile([128, NEO, 512], bf16, tag="w", name=f"w{vc}")
                nc.sync.dma_start(wt, wtet[vc])
                pt = p.tile([NCORES, 512], f32, tag="p", name=f"p{vc}")
                for eo in range(NEO):
                    nc.tensor.matmul(pt, xt[:, eo, :], wt[:, eo, :], start=(eo == 0), stop=(eo == NEO - 1))
                ot = o.tile([NCORES, 512], f32, tag="o", name=f"o{vc}")
                nc.scalar.activation(ot, pt, AF.Copy)
                nc.sync.dma_start(lg[:, vc * 512 : (vc + 1) * 512], ot)
    nc.compile()
    return nc


def _host_prep(idx, wte, wpe, ln1_w, ln1_b, attn_w, attn_b, proj_w, proj_b,
               ln2_w, ln2_b, fc_w, fc_b, fc2_w, fc2_b, lnf_w, lnf_b, nl):
    import ml_dtypes

    f = np.float32
    bf = ml_dtypes.bfloat16
    idx = np.asarray(idx)
    wte = np.asarray(wte, f)
    wpe = np.asarray(wpe, f)
    x0_all = wte[idx] + wpe[None, :T]  # [8, T, E]

    attn_w = np.asarray(attn_w, f)
    ln1_w = np.asarray(ln1_w, f)
    fc_w = np.asarray(fc_w, f)
    ln2_w = np.asarray(ln2_w, f)

    # fold ln scale into following weights; fold 1/sqrt(D) into W_q
    wqkv = attn_w * ln1_w[:, :, None]
    wqkv[:, :, :E] *= 1.0 / np.sqrt(D)
    wfc_f = fc_w * ln2_w[:, :, None]

    # biases: must be zero (true for this model)
    bqkv = np.einsum("le,lec->lc", np.asarray(ln1_b, f), attn_w) + np.asarray(attn_b, f)
    bfc = np.einsum("le,lec->lc", np.asarray(ln2_b, f), fc_w) + np.asarray(fc_b, f)
    for nm, b in [("bqkv", bqkv), ("proj_b", np.asarray(proj_b, f)),
                  ("bfc", bfc), ("fc2_b", np.asarray(fc2_b, f)),
                  ("lnf_b", np.asarray(lnf_b, f))]:
        assert np.abs(b).max() == 0.0, f"nonzero bias {nm} not supported by this kernel"

    # --- pre-tiled bf16 weight layouts (one contiguous DMA per tile load) ---
    # wqk[l, ct, p, eo, c] = Wqkv[l, eo*128+p, ct*128+c] (ct 0-7 Q, 8-15 K)
    qk = wqkv[:nl, :, : 2 * E]  # [nl, E, 2E]
    wqk_t = np.ascontiguousarray(
        qk.reshape(nl, NEO, 128, 16, 128).transpose(0, 3, 2, 1, 4).astype(bf)
    )
    # wv[l, g, p, eo, c] = Wqkv[l, eo*128+p, 2E + g*512 + c]
    vv = wqkv[:nl, :, 2 * E :]  # [nl, E, E]
    wv_t = np.ascontiguousarray(
        vv.reshape(nl, NEO, 128, 2, 512).transpose(0, 3, 2, 1, 4).astype(bf)
    )
    # wproj[l, ch, p, k, c] = Wproj[l, k*128+p, ch*512+c]
    pr = np.asarray(proj_w, f)[:nl]
    wproj_t = np.ascontiguousarray(
        pr.reshape(nl, NEO, 128, 2, 512).transpose(0, 3, 2, 1, 4).astype(bf)
    )
    # wfc[l, ct, p, eo, c] = Wfc[l, eo*128+p, ct*128+c]   (ct = slab*8 + ct')
    wfc_t = np.ascontiguousarray(
        wfc_f[:nl].reshape(nl, NEO, 128, 32, 128).transpose(0, 3, 2, 1, 4).astype(bf)
    )
    # wfc2[l, slab, ch, p, k, c] = Wfc2[l, slab*1024 + k*128 + p, ch*512 + c]
    f2 = np.asarray(fc2_w, f)[:nl]
    wfc2_t = np.ascontiguousarray(
        f2.reshape(nl, 4, NEO, 128, 2, 512).transpose(0, 1, 4, 3, 2, 5).astype(bf)
    )

    # lm_head: wtet[vc, p, eo, c] = (wte*lnf)[vshard + vc*512 + c, eo*128 + p]
    wtes = wte * np.asarray(lnf_w, f)[None, :]
    shards = []
    for cshard in range(NCORES):
        sl = wtes[cshard * VSH : min(V, (cshard + 1) * VSH)]  # [vs, E]
        pad = np.zeros((VSP, E), f)
        pad[: sl.shape[0]] = sl
        sh = pad.reshape(VSP // 512, 512, NEO, 128).transpose(0, 3, 2, 1).astype(bf)
        shards.append(np.ascontiguousarray(sh))

    return np.ascontiguousarray(x0_all, f), wqk_t, wv_t, wproj_t, wfc_t, wfc2_t, shards


def kernel(idx, wte, wpe, ln1_w, ln1_b, attn_w, attn_b, proj_w, proj_b,
           ln2_w, ln2_b, fc_w, fc_b, fc2_w, fc2_b, lnf_w, lnf_b):
    import ml_dtypes
    from concourse.bass_utils import run_bass_kernel_spmd

    x0_all, wqk_t, wv_t, wproj_t, wfc_t, wfc2_t, shards = _host_prep(
        idx, wte, wpe, ln1_w, ln1_b, attn_w, attn_b, proj_w, proj_b,
        ln2_w, ln2_b, fc_w, fc_b, fc2_w, fc2_b, lnf_w, lnf_b, NL)

    if "p1" not in _CACHE:
        _CACHE["p1"] = _build_phase1(NL)
    nc1 = _CACHE["p1"]
    in_maps = [
        {"x0": x0_all[c], "wqk": wqk_t, "wv": wv_t, "wproj": wproj_t,
         "wfc": wfc_t, "wfc2": wfc2_t}
        for c in range(NCORES)
    ]
    trace = os.environ.get("GPT_TRACE", "0") == "1"
    r1 = run_bass_kernel_spmd(nc1, in_maps, core_ids=list(range(NCORES)), trace=trace)
    _CACHE["r1"] = r1
    xall = np.stack([r1.results[c]["xlast"][0] for c in range(NCORES)])  # [8, E]
    # xt[p, eo, s] = xall[s, eo*128+p]
    xt = np.ascontiguousarray(
        xall.reshape(NCORES, NEO, 128).transpose(2, 1, 0).astype(ml_dtypes.bfloat16)
    )

    if "p2" not in _CACHE:
        _CACHE["p2"] = _build_phase2()
    nc2 = _CACHE["p2"]
    in_maps2 = [{"xt": xt, "wtet": shards[c]} for c in range(NCORES)]
    r2 = run_bass_kernel_spmd(nc2, in_maps2, core_ids=list(range(NCORES)), trace=trace)
    _CACHE["r2"] = r2

    logits = np.zeros((NCORES, 1, V), np.float32)
    for c in range(NCORES):
        w = min(V, (c + 1) * VSH) - c * VSH
        logits[:, 0, c * VSH : c * VSH + w] = r2.results[c]["lg"][:, :w]
    return logits


# revision 27
# speedup vs baseline: 1.0035x; 1.0035x over previous
"""GPT-2 (L=12, E=1024, H=16, T=1024, B=8) forward on 8 Trainium2 NeuronCores.

Strategy: data-parallel over batch (1 sequence per core) for the 12 transformer
layers; vocab-parallel lm_head (each core computes logits for a V/8 shard of the
vocabulary for all 8 sequences) as a second tiny NEFF, with the 8 last-position
hidden vectors gathered on host between the phases.

v2 (all-bf16 matmul datapath, fp32 residual/PSUM):
  - weights pre-cast to bf16 and pre-tiled on host so every weight load is one
    large contiguous DMA
  - LN output transposed via DMA-transpose (xbar) instead of PE transposes
  - attention: heads processed in pairs (even head on partitions 0-63, odd on
    64-127) so the two QK matmuls row-tile the PE array concurrently
  - softmax denominator comes free from an appended ones-column in V
    (out row 64 of the AV matmul), replacing the ones-matmul accumulation
  - causal mask applied as a post-exp zeroing affine_select on GpSimd
  - layer 11 computes Q/attention/proj/MLP only for the last 128 tokens
    (only the last position's logits are needed downstream)

Host-side preprocessing (all linear folds, no model compute):
  - embedding gather x0 = wte[idx] + wpe  (pure indexing)
  - layernorm scale folded into the following matmul weights
  - 1/sqrt(D) folded into W_q
  - wte transposed (+ lnf scale) for the lm_head
"""

import os
import sys

import numpy as np

sys.path.insert(0, "/opt/trn_rl_repo")

V, BLK, L, H, E = 50257, 1024, 12, 16, 1024
T = 1024
D = E // H  # 64
NCORES = 8
E3 = 3 * E
E4 = 4 * E
NTT = T // 128  # 8 token tiles
NEO = E // 128  # 8 embed tiles
VSH = (V + NCORES - 1) // NCORES  # 6283 vocab shard
VSP = 13 * 512  # 6656 padded shard width
NL = int(os.environ.get("GPT_NL", str(L)))
TRUNC_LAST = os.environ.get("GPT_TRUNC", "1") == "1"

_CACHE = {}

# attT column offsets for the compacted causal layout, per q-window start
def _offs(qlo):
    offs, col = [], 0
    for j in range(NTT):
        offs.append(col)
        col += T - max(j * 128, qlo)
    return offs, col


def _build_phase1(nl):
    import concourse.mybir as mybir
    import concourse.tile as tile
    from concourse import bacc

    f32 = mybir.dt.float32
    bf16 = mybir.dt.bfloat16
    AF = mybir.ActivationFunctionType
    ALU = mybir.AluOpType

    nc = bacc.Bacc("TRN2", target_bir_lowering=False)

    x0 = nc.dram_tensor("x0", [T, E], f32, kind="ExternalInput")
    wqk = nc.dram_tensor("wqk", [nl, 16, 128, NEO, 128], bf16, kind="ExternalInput")
    wv = nc.dram_tensor("wv", [nl, 2, 128, NEO, 512], bf16, kind="ExternalInput")
    wproj = nc.dram_tensor("wproj", [nl, 2, 128, NEO, 512], bf16, kind="ExternalInput")
    wfc = nc.dram_tensor("wfc", [nl, 32, 128, NEO, 128], bf16, kind="ExternalInput")
    wfc2 = nc.dram_tensor("wfc2", [nl, 4, 2, 128, NEO, 512], bf16, kind="ExternalInput")
    xlast = nc.dram_tensor("xlast", [1, E], f32, kind="ExternalOutput")

    ATT_W = 4608  # full compacted attT width per head
    VGW = 16 * 65  # V block per tt: 16 heads x (64 cols + ones col)

    with tile.TileContext(nc) as tc:
        import contextlib

        ctx = contextlib.ExitStack()
        with ctx:
            singles = ctx.enter_context(tc.tile_pool(name="singles", bufs=1))
            avs = ctx.enter_context(tc.tile_pool(name="avs", bufs=2))
            wl = ctx.enter_context(tc.tile_pool(name="wl", bufs=4))  # lhsT ct tiles
            wr = ctx.enter_context(tc.tile_pool(name="wr", bufs=3))  # rhs tiles
            hpool = ctx.enter_context(tc.tile_pool(name="hpool", bufs=2))
            stat = ctx.enter_context(tc.tile_pool(name="stat", bufs=2))
            bc = ctx.enter_context(tc.tile_pool(name="bc", bufs=2))
            dram = ctx.enter_context(tc.tile_pool(name="dram", bufs=2, space="DRAM"))
            scrA = ctx.enter_context(tc.tile_pool(name="scrA", bufs=2))
            scrB = ctx.enter_context(tc.tile_pool(name="scrB", bufs=1))
            # PSUM: pmm (evacuated mm outputs) and pqk (attention scores) are
            # separate pools so Q/K/V/MLP matmuls never block on the exp chain
            pmm = ctx.enter_context(tc.tile_pool(name="pmm", bufs=3, space="PSUM"))
            pqk = ctx.enter_context(tc.tile_pool(name="pqk", bufs=2, space="PSUM"))
            pav = ctx.enter_context(tc.tile_pool(name="pav", bufs=1, space="PSUM"))

            # ---- persistent tiles ----
            X = singles.tile([128, NTT, T], f32)  # residual [tp, tt, e]
            HT = singles.tile([128, NEO, T], bf16)  # ln-out transposed [ep, eo, t]
            AOT = singles.tile([128, NEO, T], bf16)  # attn outT [cp, co, t]
            eps_t = singles.tile([128, 1], f32)
            nc.gpsimd.memset(eps_t, 1e-5)

            # ---- load x0 ----
            x0v = x0[:, :].rearrange("(tt p) e -> p tt e", p=128)
            for tt in range(NTT):
                nc.sync.dma_start(X[:, tt, :], x0v[:, tt, :])

            def layernorm(tts, l, which):
                """LN(X[:,tt,:]) -> HT[:, :, tt*128:(tt+1)*128] via DMA transpose."""
                for tt in tts:
                    st = stat.tile([128, 2, 6], f32, tag="bnst", name=f"st{l}_{which}_{tt}")
                    for c in range(2):
                        nc.vector.bn_stats(st[:, c, :], X[:, tt, c * 512 : (c + 1) * 512])
                    mv = stat.tile([128, 2], f32, tag="bnmv", name=f"mv{l}_{which}_{tt}")
                    nc.vector.bn_aggr(mv, st)
                    rstd = stat.tile([128, 1], f32, tag="rstd", name=f"rs{l}_{which}_{tt}")
                    nc.scalar.activation(rstd, mv[:, 1:2], AF.Sqrt, bias=eps_t)
                    nc.vector.reciprocal(rstd, rstd)
                    h = hpool.tile([128, T], bf16, tag="h", name=f"h{l}_{which}_{tt}")
                    nc.vector.tensor_scalar(
                        out=h,
                        in0=X[:, tt, :],
                        scalar1=mv[:, 0:1],
                        scalar2=rstd,
                        op0=ALU.subtract,
                        op1=ALU.mult,
                    )
                    nc.sync.dma_start_transpose(HT[:, :, tt * 128 : (tt + 1) * 128], h)

            def mm_lhsw(dst_fn, wdram_ct, t_lo, t_hi, act, nm):
                """dst(ch0, w) <- W_ct.T @ HT[:, :, ch0:ch0+w] per 512 chunk."""
                wt = wl.tile([128, NEO, 128], bf16, tag="wl", name=f"wt{nm}")
                nc.sync.dma_start(wt, wdram_ct)
                for ch0 in range(t_lo, t_hi, 512):
                    w = min(512, t_hi - ch0)
                    pt = pmm.tile([128, 512], f32, tag="pmm", name=f"pt{nm}_{ch0}")
                    for eo in range(NEO):
                        nc.tensor.matmul(
                            pt[:, :w],
                            wt[:, eo, :],
                            HT[:, eo, ch0 : ch0 + w],
                            start=(eo == 0),
                            stop=(eo == NEO - 1),
                        )
                    if act is None:
                        nc.vector.tensor_copy(dst_fn(ch0, w), pt[:, :w])
                    else:
                        nc.scalar.activation(dst_fn(ch0, w), pt[:, :w], act)

            # LN1 of layer 0; later layers' LN1 is emitted inside the previous
            # layer's fc2 tail so it pipelines with the remaining MLP matmuls
            layernorm(range(NTT), 0, 0)

            for l in range(nl):
                last = TRUNC_LAST and (l == nl - 1) and (nl == L)
                qlo = T - 128 if last else 0
                tts = [NTT - 1] if last else list(range(NTT))
                offs, attw = _offs(qlo)

                # ===== attention =====
                VA = scrB.tile([128, NTT * VGW + 2 * ATT_W], bf16, tag="scrB", name=f"va{l}")
                VG = VA[:, : NTT * VGW].rearrange("p (tt h c) -> p tt h c", tt=NTT, h=16)

                def attT(slot, col, n):  # slot = head_in_pair (one pair in flight)
                    return VA[:, NTT * VGW + slot * ATT_W + col : NTT * VGW + slot * ATT_W + col + n]

                # ones columns for the free softmax denominator
                nc.gpsimd.memset(VG[:, :, :, 64:65], 1.0)

                # V for both groups first (so AV can chase QK/exp per pair)
                for g in range(2):
                    wvt = wr.tile([128, NEO, 512], bf16, tag="wr", name=f"wv{l}_{g}")
                    nc.sync.dma_start(wvt, wv[l, g])
                    for tt in range(NTT):
                        pv = pmm.tile([128, 512], f32, tag="pmm", name=f"pv{l}_{g}_{tt}")
                        for eo in range(NEO):
                            nc.tensor.matmul(
                                pv,
                                HT[:, eo, tt * 128 : (tt + 1) * 128],
                                wvt[:, eo, :],
                                start=(eo == 0),
                                stop=(eo == NEO - 1),
                            )
                        nc.scalar.activation(
                            VG[:, tt, g * 8 : (g + 1) * 8, 0:64],
                            pv.rearrange("p (h c) -> p h c", h=8),
                            AF.Copy,
                        )

                QT = scrA.tile([128, NEO, T], bf16, tag="scrA", name=f"qt{l}")
                KT = scrA.tile([128, NEO, T], bf16, tag="scrA", name=f"kt{l}")

                for p in range(NEO):  # head pair p = heads (2p, 2p+1)
                    mm_lhsw(lambda c0, w, p=p: QT[:, p, c0 : c0 + w], wqk[l, p], qlo, T, None, f"q{l}_{p}")
                    mm_lhsw(lambda c0, w, p=p: KT[:, p, c0 : c0 + w], wqk[l, 8 + p], 0, T, None, f"k{l}_{p}")

                    # --- QK for the pair (row-tiled: even rows 0-63, odd 64-127) ---
                    # consecutive k-blocks share a psum tile (and one exp) while
                    # their combined q-width fits the 2-bank tile
                    jgroups, cur, curw = [], [], 0
                    for j in range(NTT):
                        qn = T - max(j * 128, qlo)
                        if curw + qn > 1024:
                            jgroups.append(cur)
                            cur, curw = [], 0
                        cur.append((j, curw, qn))
                        curw += qn
                    jgroups.append(cur)
                    for gi, grp in enumerate(jgroups):
                        gw = sum(qn for _, _, qn in grp)
                        pe_t = pqk.tile([128, T], f32, tag="pqk", name=f"pe{l}_{p}_{gi}")
                        po_t = pqk.tile([128, T], f32, tag="pqk", name=f"po{l}_{p}_{gi}")
                        for j, lo, qn in grp:
                            qs = max(j * 128, qlo)
                            for ch0 in range(0, qn, 512):
                                w = min(512, qn - ch0)
                                nc.tensor.matmul(
                                    pe_t[:, lo + ch0 : lo + ch0 + w],
                                    KT[0:64, p, j * 128 : (j + 1) * 128],
                                    QT[0:64, p, qs + ch0 : qs + ch0 + w],
                                    start=True,
                                    stop=True,
                                )
                                nc.tensor.matmul(
                                    po_t[:, lo + ch0 : lo + ch0 + w],
                                    KT[64:128, p, j * 128 : (j + 1) * 128],
                                    QT[64:128, p, qs + ch0 : qs + ch0 + w],
                                    start=True,
                                    stop=True,
                                )
                        o0 = offs[grp[0][0]]
                        nc.scalar.activation(attT(0, o0, gw), pe_t[:, :gw], AF.Exp)
                        nc.scalar.activation(attT(1, o0, gw), po_t[:, :gw], AF.Exp)
                        for j, lo, qn in grp:
                            if j * 128 >= qlo:  # diagonal: zero strict upper triangle
                                for hh in range(2):
                                    nc.gpsimd.affine_select(
                                        out=attT(hh, offs[j], 128),
                                        in_=attT(hh, offs[j], 128),
                                        compare_op=ALU.is_ge,
                                        fill=0.0,
                                        base=0,
                                        pattern=[[1, 128]],
                                        channel_multiplier=-1,
                                    )

                    # --- AV + free denominator for both heads of the pair ---
                    for hh in range(2):
                        h_glob = 2 * p + hh
                        wd = T - qlo
                        av_s = avs.tile([65, 1024], f32, tag="avs", name=f"avs{l}_{h_glob}")
                        for ca0 in range(qlo, T, 512):
                            cw = min(512, T - ca0)
                            avp = pav.tile([65, 512], f32, tag="pav", name=f"av{l}_{h_glob}_{ca0}")
                            js = [j for j in range(NTT) if j * 128 < ca0 + cw]
                            for ji, j in enumerate(js):
                                s = max(ca0, j * 128)
                                w = ca0 + cw - s
                                nc.tensor.matmul(
                                    avp[:, s - ca0 : s - ca0 + w],
                                    VG[:, j, h_glob, :],
                                    attT(hh, offs[j] + s - max(j * 128, qlo), w),
                                    start=(ji == 0),
                                    stop=(ji == len(js) - 1),
                                    skip_group_check=True,
                                )
                            # evacuate value rows + denominator row per chunk,
                            # freeing the 1-bank PSUM accumulator quickly
                            nc.vector.tensor_copy(
                                av_s[:, ca0 - qlo : ca0 - qlo + cw], avp[:, :cw]
                            )
                        # full-tile AP (partition 0, 65 rows): reciprocal_approx_fast
                        # rejects single-partition slices at nonzero base partition
                        rden = stat.tile([65, 1024], f32, tag="rden", name=f"rd{l}_{h_glob}")
                        nc.vector.reciprocal_approx_fast(out=rden[:, :wd], in_=av_s[:, :wd])
                        rdd = dram.tile([1, 1024], f32, tag="rdd", name=f"rdd{l}_{h_glob}")
                        nc.sync.dma_start(rdd[:, :wd], rden[64:65, :wd])
                        rdb = bc.tile([64, 1024], f32, tag="rdb", name=f"rdb{l}_{h_glob}")
                        nc.sync.dma_start(rdb[:, :wd], rdd[:, :wd].to_broadcast([64, wd]))
                        co, ro2 = h_glob // 2, (h_glob % 2) * 64
                        nc.vector.tensor_tensor(
                            AOT[ro2 : ro2 + 64, co, qlo:T], av_s[0:64, :wd], rdb[:, :wd], ALU.mult
                        )

                # ===== proj + residual =====
                wp0 = wr.tile([128, NEO, 512], bf16, tag="wr", name=f"wp0{l}")
                wp1 = wr.tile([128, NEO, 512], bf16, tag="wr", name=f"wp1{l}")
                nc.sync.dma_start(wp0, wproj[l, 0])
                nc.sync.dma_start(wp1, wproj[l, 1])
                for tt in tts:
                    for chi, wpt in ((0, wp0), (1, wp1)):
                        pp = pmm.tile([128, 512], f32, tag="pmm", name=f"pp{l}_{tt}_{chi}")
                        for k in range(NEO):
                            nc.tensor.matmul(
                                pp,
                                AOT[:, k, tt * 128 : (tt + 1) * 128],
                                wpt[:, k, :],
                                start=(k == 0),
                                stop=(k == NEO - 1),
                            )
                        xs = X[:, tt, chi * 512 : chi * 512 + 512]
                        nc.vector.tensor_tensor(xs, xs, pp, ALU.add)
                    # LN2 for this token tile rides the proj loop so fc1 never
                    # waits on a serialized LN chain after proj completes
                    layernorm([tt], l, 1)

                # ===== mlp =====  (LN2 already emitted inside the proj loop)
                FC2A = scrB.tile([128, NTT, T], f32, tag="scrB", name=f"fc2a{l}")
                for slab in range(4):
                    H1T = scrA.tile([128, NEO, T], bf16, tag="scrA", name=f"h1t{l}_{slab}")
                    for ct in range(NEO):
                        mm_lhsw(
                            lambda c0, w, ct=ct, H1T=H1T: H1T[:, ct, c0 : c0 + w],
                            wfc[l, slab * 8 + ct],
                            qlo,
                            T,
                            AF.Gelu_apprx_tanh,
                            f"f{l}_{slab}_{ct}",
                        )
                    w20 = wr.tile([128, NEO, 512], bf16, tag="wr", name=f"w20{l}_{slab}")
                    w21 = wr.tile([128, NEO, 512], bf16, tag="wr", name=f"w21{l}_{slab}")
                    nc.sync.dma_start(w20, wfc2[l, slab, 0])
                    nc.sync.dma_start(w21, wfc2[l, slab, 1])
                    for tt in tts:
                        for chi, w2t in ((0, w20), (1, w21)):
                            p2 = pmm.tile([128, 512], f32, tag="pmm", name=f"p2{l}_{slab}_{tt}_{chi}")
                            for k in range(NEO):
                                nc.tensor.matmul(
                                    p2,
                                    H1T[:, k, tt * 128 : (tt + 1) * 128],
                                    w2t[:, k, :],
                                    start=(k == 0),
                                    stop=(k == NEO - 1),
                                )
                            sl = slice(chi * 512, chi * 512 + 512)
                            if slab == 0:
                                nc.vector.tensor_tensor(FC2A[:, tt, sl], X[:, tt, sl], p2, ALU.add)
                            elif slab < 3:
                                nc.vector.tensor_tensor(FC2A[:, tt, sl], FC2A[:, tt, sl], p2, ALU.add)
                            else:
                                nc.vector.tensor_tensor(X[:, tt, sl], FC2A[:, tt, sl], p2, ALU.add)
                        if slab == 3 and l + 1 < nl:
                            # next layer's LN1 for this token tile rides the
                            # fc2 tail so the layer boundary never idles PE
                            layernorm([tt], l + 1, 0)

            # ===== final layernorm on last token tile, emit last row =====
            st = stat.tile([128, 2, 6], f32, tag="bnst", name="stf")
            for c in range(2):
                nc.vector.bn_stats(st[:, c, :], X[:, NTT - 1, c * 512 : (c + 1) * 512])
            mv = stat.tile([128, 2], f32, tag="bnmv", name="mvf")
            nc.vector.bn_aggr(mv, st)
            rstd = stat.tile([128, 1], f32, tag="rstd", name="rsf")
            nc.scalar.activation(rstd, mv[:, 1:2], AF.Sqrt, bias=eps_t)
            nc.vector.reciprocal(rstd, rstd)
            xn = hpool.tile([128, T], f32, tag="xn", name="xnf", bufs=1)
            nc.vector.tensor_scalar(
                out=xn,
                in0=X[:, NTT - 1, :],
                scalar1=mv[:, 0:1],
                scalar2=rstd,
                op0=ALU.subtract,
                op1=ALU.mult,
            )
            nc.sync.dma_start(xlast[:, :], xn[127:128, :])

    nc.compile()
    return nc


def _build_phase2():
    import concourse.mybir as mybir
    import concourse.tile as tile
    from concourse import bacc

    f32 = mybir.dt.float32
    bf16 = mybir.dt.bfloat16
    AF = mybir.ActivationFunctionType

    nc = bacc.Bacc("TRN2", target_bir_lowering=False)
    xt_d = nc.dram_tensor("xt", [128, NEO, NCORES], bf16, kind="ExternalInput")
    wtet = nc.dram_tensor("wtet", [VSP // 512, 128, NEO, 512], bf16, kind="ExternalInput")
    lg = nc.dram_tensor("lg", [NCORES, VSP], f32, kind="ExternalOutput")

    with tile.TileContext(nc) as tc:
        with (
            tc.tile_pool(name="s", bufs=1) as s,
            tc.tile_pool(name="w", bufs=4) as w,
            tc.tile_pool(name="o", bufs=4) as o,
            tc.tile_pool(name="p", bufs=4, space="PSUM") as p,
        ):
            xt = s.tile([128, NEO, NCORES], bf16)
            nc.sync.dma_start(xt, xt_d[:, :, :])
            for vc in range(VSP // 512):
                wt = w.tile([128, NEO, 512], bf16, tag="w", name=f"w{vc}")
                nc.sync.dma_start(wt, wtet[vc])
                pt = p.tile([NCORES, 512], f32, tag="p", name=f"p{vc}")
                for eo in range(NEO):
                    nc.tensor.matmul(pt, xt[:, eo, :], wt[:, eo, :], start=(eo == 0), stop=(eo == NEO - 1))
                ot = o.tile([NCORES, 512], f32, tag="o", name=f"o{vc}")
                nc.scalar.activation(ot, pt, AF.Copy)
                nc.sync.dma_start(lg[:, vc * 512 : (vc + 1) * 512], ot)
    nc.compile()
    return nc


def _host_prep(idx, wte, wpe, ln1_w, ln1_b, attn_w, attn_b, proj_w, proj_b,
               ln2_w, ln2_b, fc_w, fc_b, fc2_w, fc2_b, lnf_w, lnf_b, nl):
    import ml_dtypes

    f = np.float32
    bf = ml_dtypes.bfloat16
    idx = np.asarray(idx)
    wte = np.asarray(wte, f)
    wpe = np.asarray(wpe, f)
    x0_all = wte[idx] + wpe[None, :T]  # [8, T, E]

    attn_w = np.asarray(attn_w, f)
    ln1_w = np.asarray(ln1_w, f)
    fc_w = np.asarray(fc_w, f)
    ln2_w = np.asarray(ln2_w, f)

    # fold ln scale into following weights; fold 1/sqrt(D) into W_q
    wqkv = attn_w * ln1_w[:, :, None]
    wqkv[:, :, :E] *= 1.0 / np.sqrt(D)
    wfc_f = fc_w * ln2_w[:, :, None]

    # biases: must be zero (true for this model)
    bqkv = np.einsum("le,lec->lc", np.asarray(ln1_b, f), attn_w) + np.asarray(attn_b, f)
    bfc = np.einsum("le,lec->lc", np.asarray(ln2_b, f), fc_w) + np.asarray(fc_b, f)
    for nm, b in [("bqkv", bqkv), ("proj_b", np.asarray(proj_b, f)),
                  ("bfc", bfc), ("fc2_b", np.asarray(fc2_b, f)),
                  ("lnf_b", np.asarray(lnf_b, f))]:
        assert np.abs(b).max() == 0.0, f"nonzero bias {nm} not supported by this kernel"

    # --- pre-tiled bf16 weight layouts (one contiguous DMA per tile load) ---
    # wqk[l, ct, p, eo, c] = Wqkv[l, eo*128+p, ct*128+c] (ct 0-7 Q, 8-15 K)
    qk = wqkv[:nl, :, : 2 * E]  # [nl, E, 2E]
    wqk_t = np.ascontiguousarray(
        qk.reshape(nl, NEO, 128, 16, 128).transpose(0, 3, 2, 1, 4).astype(bf)
    )
    # wv[l, g, p, eo, c] = Wqkv[l, eo*128+p, 2E + g*512 + c]
    vv = wqkv[:nl, :, 2 * E :]  # [nl, E, E]
    wv_t = np.ascontiguousarray(
        vv.reshape(nl, NEO, 128, 2, 512).transpose(0, 3, 2, 1, 4).astype(bf)
    )
    # wproj[l, ch, p, k, c] = Wproj[l, k*128+p, ch*512+c]
    pr = np.asarray(proj_w, f)[:nl]
    wproj_t = np.ascontiguousarray(
        pr.reshape(nl, NEO, 128, 2, 512).transpose(0, 3, 2, 1, 4).astype(bf)
    )
    # wfc[l, ct, p, eo, c] = Wfc[l, eo*128+p, ct*128+c]   (ct = slab*8 + ct')
    wfc_t = np.ascontiguousarray(
        wfc_f[:nl].reshape(nl, NEO, 128, 32, 128).transpose(0, 3, 2, 1, 4).astype(bf)
    )
    # wfc2[l, slab, ch, p, k, c] = Wfc2[l, slab*1024 + k*128 + p, ch*512 + c]
    f2 = np.asarray(fc2_w, f)[:nl]
    wfc2_t = np.ascontiguousarray(
        f2.reshape(nl, 4, NEO, 128, 2, 512).transpose(0, 1, 4, 3, 2, 5).astype(bf)
    )

    # lm_head: wtet[vc, p, eo, c] = (wte*lnf)[vshard + vc*512 + c, eo*128 + p]
    wtes = wte * np.asarray(lnf_w, f)[None, :]
    shards = []
    for cshard in range(NCORES):
        sl = wtes[cshard * VSH : min(V, (cshard + 1) * VSH)]  # [vs, E]
        pad = np.zeros((VSP, E), f)
        pad[: sl.shape[0]] = sl
        sh = pad.reshape(VSP // 512, 512, NEO, 128).transpose(0, 3, 2, 1).astype(bf)
        shards.append(np.ascontiguousarray(sh))

    return np.ascontiguousarray(x0_all, f), wqk_t, wv_t, wproj_t, wfc_t, wfc2_t, shards


def kernel(idx, wte, wpe, ln1_w, ln1_b, attn_w, attn_b, proj_w, proj_b,
           ln2_w, ln2_b, fc_w, fc_b, fc2_w, fc2_b, lnf_w, lnf_b):
    import ml_dtypes
    from concourse.bass_utils import run_bass_kernel_spmd

    x0_all, wqk_t, wv_t, wproj_t, wfc_t, wfc2_t, shards = _host_prep(
        idx, wte, wpe, ln1_w, ln1_b, attn_w, attn_b, proj_w, proj_b,
        ln2_w, ln2_b, fc_w, fc_b, fc2_w, fc2_b, lnf_w, lnf_b, NL)

    if "p1" not in _CACHE:
        _CACHE["p1"] = _build_phase1(NL)
    nc1 = _CACHE["p1"]
    in_maps = [
        {"x0": x0_all[c], "wqk": wqk_t, "wv": wv_t, "wproj": wproj_t,
         "wfc": wfc_t, "wfc2": wfc2_t}
        for c in range(NCORES)
    ]
    trace = os.environ.get("GPT_TRACE", "0") == "1"
    r1 = run_bass_kernel_spmd(nc1, in_maps, core_ids=list(range(NCORES)), trace=trace)
    _CACHE["r1"] = r1
    xall = np.stack([r1.results[c]["xlast"][0] for c in range(NCORES)])  # [8, E]
    # xt[p, eo, s] = xall[s, eo*128+p]
    xt = np.ascontiguousarray(
        xall.reshape(NCORES, NEO, 128).transpose(2, 1, 0).astype(ml_dtypes.bfloat16)
    )

    if "p2" not in _CACHE:
        _CACHE["p2"] = _build_phase2()
    nc2 = _CACHE["p2"]
    in_maps2 = [{"xt": xt, "wtet": shards[c]} for c in range(NCORES)]
    r2 = run_bass_kernel_spmd(nc2, in_maps2, core_ids=list(range(NCORES)), trace=trace)
    _CACHE["r2"] = r2

    logits = np.zeros((NCORES, 1, V), np.float32)
    for c in range(NCORES):
        w = min(V, (c + 1) * VSH) - c * VSH
        logits[:, 0, c * VSH : c * VSH + w] = r2.results[c]["lg"][:, :w]
    return logits


# revision 29
# speedup vs baseline: 1.0906x; 1.0868x over previous
"""GPT-2 (L=12, E=1024, H=16, T=1024, B=8) forward on 8 Trainium2 NeuronCores.

Strategy: data-parallel over batch (1 sequence per core) for the 12 transformer
layers; vocab-parallel lm_head (each core computes logits for a V/8 shard of the
vocabulary for all 8 sequences) as a second tiny NEFF, with the 8 last-position
hidden vectors gathered on host between the phases.

v2 (all-bf16 matmul datapath, fp32 residual/PSUM):
  - weights pre-cast to bf16 and pre-tiled on host so every weight load is one
    large contiguous DMA
  - LN output transposed via DMA-transpose (xbar) instead of PE transposes
  - attention: heads processed in pairs (even head on partitions 0-63, odd on
    64-127) so the two QK matmuls row-tile the PE array concurrently
  - softmax denominator comes free from an appended ones-column in V
    (out row 64 of the AV matmul), replacing the ones-matmul accumulation
  - causal mask applied as a post-exp zeroing affine_select on GpSimd
  - layer 11 computes Q/attention/proj/MLP only for the last 128 tokens
    (only the last position's logits are needed downstream)

Host-side preprocessing (all linear folds, no model compute):
  - embedding gather x0 = wte[idx] + wpe  (pure indexing)
  - layernorm scale folded into the following matmul weights
  - 1/sqrt(D) folded into W_q
  - wte transposed (+ lnf scale) for the lm_head
"""

import os
import sys

import numpy as np

sys.path.insert(0, "/opt/trn_rl_repo")

V, BLK, L, H, E = 50257, 1024, 12, 16, 1024
T = 1024
D = E // H  # 64
NCORES = 8
E3 = 3 * E
E4 = 4 * E
NTT = T // 128  # 8 token tiles
NEO = E // 128  # 8 embed tiles
VSH = (V + NCORES - 1) // NCORES  # 6283 vocab shard
VSP = 13 * 512  # 6656 padded shard width
NL = int(os.environ.get("GPT_NL", str(L)))
TRUNC_LAST = os.environ.get("GPT_TRUNC", "1") == "1"

_CACHE = {}

# attT column offsets for the compacted causal layout, per q-window start
def _offs(qlo):
    offs, col = [], 0
    for j in range(NTT):
        offs.append(col)
        col += T - max(j * 128, qlo)
    return offs, col


def _build_phase1(nl):
    import concourse.mybir as mybir
    import concourse.tile as tile
    from concourse import bacc

    f32 = mybir.dt.float32
    bf16 = mybir.dt.bfloat16
    AF = mybir.ActivationFunctionType
    ALU = mybir.AluOpType

    nc = bacc.Bacc("TRN2", target_bir_lowering=False)

    x0 = nc.dram_tensor("x0", [T, E], f32, kind="ExternalInput")
    wqk = nc.dram_tensor("wqk", [nl, 16, 128, NEO, 128], bf16, kind="ExternalInput")
    wv = nc.dram_tensor("wv", [nl, 2, 128, NEO, 512], bf16, kind="ExternalInput")
    wproj = nc.dram_tensor("wproj", [nl, 2, 128, NEO, 512], bf16, kind="ExternalInput")
    wfc = nc.dram_tensor("wfc", [nl, 32, 128, NEO, 128], bf16, kind="ExternalInput")
    wfc2 = nc.dram_tensor("wfc2", [nl, 4, 2, 128, NEO, 512], bf16, kind="ExternalInput")
    xlast = nc.dram_tensor("xlast", [1, E], f32, kind="ExternalOutput")

    ATT_W = 4608  # full compacted attT width per head
    VGW = 16 * 65  # V block per tt: 16 heads x (64 cols + ones col)

    with tile.TileContext(nc) as tc:
        import contextlib

        ctx = contextlib.ExitStack()
        with ctx:
            singles = ctx.enter_context(tc.tile_pool(name="singles", bufs=1))
            avs = ctx.enter_context(tc.tile_pool(name="avs", bufs=2))
            wl = ctx.enter_context(tc.tile_pool(name="wl", bufs=3))  # lhsT ct tiles
            wr = ctx.enter_context(tc.tile_pool(name="wr", bufs=3))  # rhs tiles
            hpool = ctx.enter_context(tc.tile_pool(name="hpool", bufs=2))
            stat = ctx.enter_context(tc.tile_pool(name="stat", bufs=2))
            bc = ctx.enter_context(tc.tile_pool(name="bc", bufs=2))
            dram = ctx.enter_context(tc.tile_pool(name="dram", bufs=2, space="DRAM"))
            scrA = ctx.enter_context(tc.tile_pool(name="scrA", bufs=2))
            scrB = ctx.enter_context(tc.tile_pool(name="scrB", bufs=1))
            # PSUM: pmm (evacuated mm outputs) and pqk (attention scores) are
            # separate pools so Q/K/V/MLP matmuls never block on the exp chain
            pmm = ctx.enter_context(tc.tile_pool(name="pmm", bufs=2, space="PSUM"))
            pqk = ctx.enter_context(tc.tile_pool(name="pqk", bufs=2, space="PSUM"))
            pav = ctx.enter_context(tc.tile_pool(name="pav", bufs=1, space="PSUM"))

            # ---- persistent tiles ----
            X = singles.tile([128, NTT, T], f32)  # residual [tp, tt, e]
            HT = singles.tile([128, NEO, T], bf16)  # ln-out transposed [ep, eo, t]
            AOT = singles.tile([128, NEO, T], bf16)  # attn outT [cp, co, t]
            eps_t = singles.tile([128, 1], f32)
            nc.gpsimd.memset(eps_t, 1e-5)

            # ---- load x0 ----
            x0v = x0[:, :].rearrange("(tt p) e -> p tt e", p=128)
            for tt in range(NTT):
                nc.sync.dma_start(X[:, tt, :], x0v[:, tt, :])

            def layernorm(tts, l, which):
                """LN(X[:,tt,:]) -> HT[:, :, tt*128:(tt+1)*128] via DMA transpose."""
                for tt in tts:
                    st = stat.tile([128, 2, 6], f32, tag="bnst", name=f"st{l}_{which}_{tt}")
                    for c in range(2):
                        nc.vector.bn_stats(st[:, c, :], X[:, tt, c * 512 : (c + 1) * 512])
                    mv = stat.tile([128, 2], f32, tag="bnmv", name=f"mv{l}_{which}_{tt}")
                    nc.vector.bn_aggr(mv, st)
                    rstd = stat.tile([128, 1], f32, tag="rstd", name=f"rs{l}_{which}_{tt}")
                    nc.scalar.activation(rstd, mv[:, 1:2], AF.Sqrt, bias=eps_t)
                    nc.vector.reciprocal(rstd, rstd)
                    h = hpool.tile([128, T], bf16, tag="h", name=f"h{l}_{which}_{tt}")
                    nc.vector.tensor_scalar(
                        out=h,
                        in0=X[:, tt, :],
                        scalar1=mv[:, 0:1],
                        scalar2=rstd,
                        op0=ALU.subtract,
                        op1=ALU.mult,
                    )
                    nc.sync.dma_start_transpose(HT[:, :, tt * 128 : (tt + 1) * 128], h)

            def mm_lhsw(dst_fn, wdram_ct, t_lo, t_hi, act, nm):
                """dst(ch0, w) <- W_ct.T @ HT[:, :, ch0:ch0+w] per 512 chunk."""
                wt = wl.tile([128, NEO, 128], bf16, tag="wl", name=f"wt{nm}")
                nc.sync.dma_start(wt, wdram_ct)
                for ch0 in range(t_lo, t_hi, 512):
                    w = min(512, t_hi - ch0)
                    pt = pmm.tile([128, 512], f32, tag="pmm", name=f"pt{nm}_{ch0}")
                    for eo in range(NEO):
                        nc.tensor.matmul(
                            pt[:, :w],
                            wt[:, eo, :],
                            HT[:, eo, ch0 : ch0 + w],
                            start=(eo == 0),
                            stop=(eo == NEO - 1),
                        )
                    if act is None:
                        nc.vector.tensor_copy(dst_fn(ch0, w), pt[:, :w])
                    else:
                        nc.scalar.activation(dst_fn(ch0, w), pt[:, :w], act)

            # LN1 of layer 0; later layers' LN1 is emitted inside the previous
            # layer's fc2 tail so it pipelines with the remaining MLP matmuls
            layernorm(range(NTT), 0, 0)

            for l in range(nl):
                last = TRUNC_LAST and (l == nl - 1) and (nl == L)
                qlo = T - 128 if last else 0
                tts = [NTT - 1] if last else list(range(NTT))
                offs, attw = _offs(qlo)

                # ===== attention =====
                VA = scrB.tile([128, NTT * VGW + 2 * ATT_W], bf16, tag="scrB", name=f"va{l}")
                VG = VA[:, : NTT * VGW].rearrange("p (tt h c) -> p tt h c", tt=NTT, h=16)

                def attT(slot, col, n):  # slot = head_in_pair (one pair in flight)
                    return VA[:, NTT * VGW + slot * ATT_W + col : NTT * VGW + slot * ATT_W + col + n]

                # ones columns for the free softmax denominator
                nc.gpsimd.memset(VG[:, :, :, 64:65], 1.0)

                # V for both groups first (so AV can chase QK/exp per pair)
                for g in range(2):
                    wvt = wr.tile([128, NEO, 512], bf16, tag="wr", name=f"wv{l}_{g}")
                    nc.sync.dma_start(wvt, wv[l, g])
                    for tt in range(NTT):
                        pv = pmm.tile([128, 512], f32, tag="pmm", name=f"pv{l}_{g}_{tt}")
                        for eo in range(NEO):
                            nc.tensor.matmul(
                                pv,
                                HT[:, eo, tt * 128 : (tt + 1) * 128],
                                wvt[:, eo, :],
                                start=(eo == 0),
                                stop=(eo == NEO - 1),
                            )
                        nc.scalar.activation(
                            VG[:, tt, g * 8 : (g + 1) * 8, 0:64],
                            pv.rearrange("p (h c) -> p h c", h=8),
                            AF.Copy,
                        )

                QT = scrA.tile([128, NEO, T], bf16, tag="scrA", name=f"qt{l}")
                KT = scrA.tile([128, NEO, T], bf16, tag="scrA", name=f"kt{l}")

                for p in range(NEO):  # head pair p = heads (2p, 2p+1)
                    mm_lhsw(lambda c0, w, p=p: QT[:, p, c0 : c0 + w], wqk[l, p], qlo, T, None, f"q{l}_{p}")
                    mm_lhsw(lambda c0, w, p=p: KT[:, p, c0 : c0 + w], wqk[l, 8 + p], 0, T, None, f"k{l}_{p}")

                    # --- QK for the pair (row-tiled: even rows 0-63, odd 64-127) ---
                    # consecutive k-blocks share a psum tile (and one exp) while
                    # their combined q-width fits the 2-bank tile
                    jgroups, cur, curw = [], [], 0
                    for j in range(NTT):
                        qn = T - max(j * 128, qlo)
                        if curw + qn > 1024:
                            jgroups.append(cur)
                            cur, curw = [], 0
                        cur.append((j, curw, qn))
                        curw += qn
                    jgroups.append(cur)
                    for gi, grp in enumerate(jgroups):
                        gw = sum(qn for _, _, qn in grp)
                        pe_t = pqk.tile([128, T], f32, tag="pqk", name=f"pe{l}_{p}_{gi}")
                        po_t = pqk.tile([128, T], f32, tag="pqk", name=f"po{l}_{p}_{gi}")
                        for j, lo, qn in grp:
                            qs = max(j * 128, qlo)
                            for ch0 in range(0, qn, 512):
                                w = min(512, qn - ch0)
                                nc.tensor.matmul(
                                    pe_t[:, lo + ch0 : lo + ch0 + w],
                                    KT[0:64, p, j * 128 : (j + 1) * 128],
                                    QT[0:64, p, qs + ch0 : qs + ch0 + w],
                                    start=True,
                                    stop=True,
                                )
                                nc.tensor.matmul(
                                    po_t[:, lo + ch0 : lo + ch0 + w],
                                    KT[64:128, p, j * 128 : (j + 1) * 128],
                                    QT[64:128, p, qs + ch0 : qs + ch0 + w],
                                    start=True,
                                    stop=True,
                                )
                        o0 = offs[grp[0][0]]
                        nc.scalar.activation(attT(0, o0, gw), pe_t[:, :gw], AF.Exp)
                        nc.scalar.activation(attT(1, o0, gw), po_t[:, :gw], AF.Exp)
                        for j, lo, qn in grp:
                            if j * 128 >= qlo:  # diagonal: zero strict upper triangle
                                for hh in range(2):
                                    nc.gpsimd.affine_select(
                                        out=attT(hh, offs[j], 128),
                                        in_=attT(hh, offs[j], 128),
                                        compare_op=ALU.is_ge,
                                        fill=0.0,
                                        base=0,
                                        pattern=[[1, 128]],
                                        channel_multiplier=-1,
                                    )

                    # --- AV + free denominator for both heads of the pair ---
                    for hh in range(2):
                        h_glob = 2 * p + hh
                        wd = T - qlo
                        avp = pav.tile([65, 1024], f32, tag="pav", name=f"av{l}_{h_glob}")
                        for ca0 in range(qlo, T, 512):
                            cw = min(512, T - ca0)
                            js = [j for j in range(NTT) if j * 128 < ca0 + cw]
                            for ji, j in enumerate(js):
                                s = max(ca0, j * 128)
                                w = ca0 + cw - s
                                nc.tensor.matmul(
                                    avp[:, s - qlo : s - qlo + w],
                                    VG[:, j, h_glob, :],
                                    attT(hh, offs[j] + s - max(j * 128, qlo), w),
                                    start=(ji == 0),
                                    stop=(ji == len(js) - 1),
                                    skip_group_check=True,
                                )
                        # one copy evacuates value rows + denominator row,
                        # freeing the PSUM accumulator for the next head fast
                        av_s = avs.tile([65, 1024], f32, tag="avs", name=f"avs{l}_{h_glob}")
                        nc.vector.tensor_copy(av_s[:, :wd], avp[:, :wd])
                        # full-tile AP (partition 0, 65 rows): reciprocal_approx_fast
                        # rejects single-partition slices at nonzero base partition
                        rden = stat.tile([65, 1024], f32, tag="rden", name=f"rd{l}_{h_glob}")
                        nc.vector.reciprocal_approx_fast(out=rden[:, :wd], in_=av_s[:, :wd])
                        rdd = dram.tile([1, 1024], f32, tag="rdd", name=f"rdd{l}_{h_glob}")
                        nc.sync.dma_start(rdd[:, :wd], rden[64:65, :wd])
                        rdb = bc.tile([64, 1024], f32, tag="rdb", name=f"rdb{l}_{h_glob}")
                        nc.sync.dma_start(rdb[:, :wd], rdd[:, :wd].to_broadcast([64, wd]))
                        co, ro2 = h_glob // 2, (h_glob % 2) * 64
                        nc.vector.tensor_tensor(
                            AOT[ro2 : ro2 + 64, co, qlo:T], av_s[0:64, :wd], rdb[:, :wd], ALU.mult
                        )

                # ===== proj + residual =====
                wp0 = wr.tile([128, NEO, 512], bf16, tag="wr", name=f"wp0{l}")
                wp1 = wr.tile([128, NEO, 512], bf16, tag="wr", name=f"wp1{l}")
                nc.sync.dma_start(wp0, wproj[l, 0])
                nc.sync.dma_start(wp1, wproj[l, 1])
                for tt in tts:
                    for chi, wpt in ((0, wp0), (1, wp1)):
                        pp = pmm.tile([128, 512], f32, tag="pmm", name=f"pp{l}_{tt}_{chi}")
                        for k in range(NEO):
                            nc.tensor.matmul(
                                pp,
                                AOT[:, k, tt * 128 : (tt + 1) * 128],
                                wpt[:, k, :],
                                start=(k == 0),
                                stop=(k == NEO - 1),
                            )
                        xs = X[:, tt, chi * 512 : chi * 512 + 512]
                        nc.vector.tensor_tensor(xs, xs, pp, ALU.add)
                    # LN2 for this token tile rides the proj loop so fc1 never
                    # waits on a serialized LN chain after proj completes
                    layernorm([tt], l, 1)

                # ===== mlp =====  (LN2 already emitted inside the proj loop)
                FC2A = scrB.tile([128, NTT, T], f32, tag="scrB", name=f"fc2a{l}")
                for slab in range(4):
                    H1T = scrA.tile([128, NEO, T], bf16, tag="scrA", name=f"h1t{l}_{slab}")
                    for ct in range(NEO):
                        mm_lhsw(
                            lambda c0, w, ct=ct, H1T=H1T: H1T[:, ct, c0 : c0 + w],
                            wfc[l, slab * 8 + ct],
                            qlo,
                            T,
                            AF.Gelu_apprx_tanh,
                            f"f{l}_{slab}_{ct}",
                        )
                    w20 = wr.tile([128, NEO, 512], bf16, tag="wr", name=f"w20{l}_{slab}")
                    w21 = wr.tile([128, NEO, 512], bf16, tag="wr", name=f"w21{l}_{slab}")
                    nc.sync.dma_start(w20, wfc2[l, slab, 0])
                    nc.sync.dma_start(w21, wfc2[l, slab, 1])
                    for tt in tts:
                        for chi, w2t in ((0, w20), (1, w21)):
                            p2 = pmm.tile([128, 512], f32, tag="pmm", name=f"p2{l}_{slab}_{tt}_{chi}")
                            for k in range(NEO):
                                nc.tensor.matmul(
                                    p2,
                                    H1T[:, k, tt * 128 : (tt + 1) * 128],
                                    w2t[:, k, :],
                                    start=(k == 0),
                                    stop=(k == NEO - 1),
                                )
                            sl = slice(chi * 512, chi * 512 + 512)
                            if slab == 0:
                                nc.vector.tensor_tensor(FC2A[:, tt, sl], X[:, tt, sl], p2, ALU.add)
                            elif slab < 3:
                                nc.vector.tensor_tensor(FC2A[:, tt, sl], FC2A[:, tt, sl], p2, ALU.add)
                            else:
                                nc.vector.tensor_tensor(X[:, tt, sl], FC2A[:, tt, sl], p2, ALU.add)
                        if slab == 3 and l + 1 < nl:
                            # next layer's LN1 for this token tile rides the
                            # fc2 tail so the layer boundary never idles PE
                            layernorm([tt], l + 1, 0)

            # ===== final layernorm on last token tile, emit last row =====
            st = stat.tile([128, 2, 6], f32, tag="bnst", name="stf")
            for c in range(2):
                nc.vector.bn_stats(st[:, c, :], X[:, NTT - 1, c * 512 : (c + 1) * 512])
            mv = stat.tile([128, 2], f32, tag="bnmv", name="mvf")
            nc.vector.bn_aggr(mv, st)
            rstd = stat.tile([128, 1], f32, tag="rstd", name="rsf")
            nc.scalar.activation(rstd, mv[:, 1:2], AF.Sqrt, bias=eps_t)
            nc.vector.reciprocal(rstd, rstd)
            xn = hpool.tile([128, T], f32, tag="xn", name="xnf", bufs=1)
            nc.vector.tensor_scalar(
                out=xn,
                in0=X[:, NTT - 1, :],
                scalar1=mv[:, 0:1],
                scalar2=rstd,
                op0=ALU.subtract,
                op1=ALU.mult,
            )
            nc.sync.dma_start(xlast[:, :], xn[127:128, :])

    nc.compile()
    return nc


def _build_phase2():
    import concourse.mybir as mybir
    import concourse.tile as tile
    from concourse import bacc

    f32 = mybir.dt.float32
    bf16 = mybir.dt.bfloat16
    AF = mybir.ActivationFunctionType

    nc = bacc.Bacc("TRN2", target_bir_lowering=False)
    xt_d = nc.dram_tensor("xt", [128, NEO, NCORES], bf16, kind="ExternalInput")
    wtet = nc.dram_tensor("wtet", [VSP // 512, 128, NEO, 512], bf16, kind="ExternalInput")
    lg = nc.dram_tensor("lg", [NCORES, VSP], f32, kind="ExternalOutput")

    with tile.TileContext(nc) as tc:
        with (
            tc.tile_pool(name="s", bufs=1) as s,
            tc.tile_pool(name="w", bufs=6) as w,
            tc.tile_pool(name="o", bufs=4) as o,
            tc.tile_pool(name="p", bufs=4, space="PSUM") as p,
        ):
            xt = s.tile([128, NEO, NCORES], bf16)
            nc.sync.dma_start(xt, xt_d[:, :, :])
            for vc in range(VSP // 512):
                wt = w.tile([128, NEO, 512], bf16, tag="w", name=f"w{vc}")
                eng = nc.sync if vc % 2 == 0 else nc.scalar
                eng.dma_start(wt, wtet[vc])
                pt = p.tile([NCORES, 512], f32, tag="p", name=f"p{vc}")
                for eo in range(NEO):
                    nc.tensor.matmul(pt, xt[:, eo, :], wt[:, eo, :], start=(eo == 0), stop=(eo == NEO - 1))
                ot = o.tile([NCORES, 512], f32, tag="o", name=f"o{vc}")
                nc.scalar.activation(ot, pt, AF.Copy)
                nc.sync.dma_start(lg[:, vc * 512 : (vc + 1) * 512], ot)
    nc.compile()
    return nc


def _host_prep(idx, wte, wpe, ln1_w, ln1_b, attn_w, attn_b, proj_w, proj_b,
               ln2_w, ln2_b, fc_w, fc_b, fc2_w, fc2_b, lnf_w, lnf_b, nl):
    import ml_dtypes

    f = np.float32
    bf = ml_dtypes.bfloat16
    idx = np.asarray(idx)
    wte = np.asarray(wte, f)
    wpe = np.asarray(wpe, f)
    x0_all = wte[idx] + wpe[None, :T]  # [8, T, E]

    attn_w = np.asarray(attn_w, f)
    ln1_w = np.asarray(ln1_w, f)
    fc_w = np.asarray(fc_w, f)
    ln2_w = np.asarray(ln2_w, f)

    # fold ln scale into following weights; fold 1/sqrt(D) into W_q
    wqkv = attn_w * ln1_w[:, :, None]
    wqkv[:, :, :E] *= 1.0 / np.sqrt(D)
    wfc_f = fc_w * ln2_w[:, :, None]

    # biases: must be zero (true for this model)
    bqkv = np.einsum("le,lec->lc", np.asarray(ln1_b, f), attn_w) + np.asarray(attn_b, f)
    bfc = np.einsum("le,lec->lc", np.asarray(ln2_b, f), fc_w) + np.asarray(fc_b, f)
    for nm, b in [("bqkv", bqkv), ("proj_b", np.asarray(proj_b, f)),
                  ("bfc", bfc), ("fc2_b", np.asarray(fc2_b, f)),
                  ("lnf_b", np.asarray(lnf_b, f))]:
        assert np.abs(b).max() == 0.0, f"nonzero bias {nm} not supported by this kernel"

    # --- pre-tiled bf16 weight layouts (one contiguous DMA per tile load) ---
    # wqk[l, ct, p, eo, c] = Wqkv[l, eo*128+p, ct*128+c] (ct 0-7 Q, 8-15 K)
    qk = wqkv[:nl, :, : 2 * E]  # [nl, E, 2E]
    wqk_t = np.ascontiguousarray(
        qk.reshape(nl, NEO, 128, 16, 128).transpose(0, 3, 2, 1, 4).astype(bf)
    )
    # wv[l, g, p, eo, c] = Wqkv[l, eo*128+p, 2E + g*512 + c]
    vv = wqkv[:nl, :, 2 * E :]  # [nl, E, E]
    wv_t = np.ascontiguousarray(
        vv.reshape(nl, NEO, 128, 2, 512).transpose(0, 3, 2, 1, 4).astype(bf)
    )
    # wproj[l, ch, p, k, c] = Wproj[l, k*128+p, ch*512+c]
    pr = np.asarray(proj_w, f)[:nl]
    wproj_t = np.ascontiguousarray(
        pr.reshape(nl, NEO, 128, 2, 512).transpose(0, 3, 2, 1, 4).astype(bf)
    )
    # wfc[l, ct, p, eo, c] = Wfc[l, eo*128+p, ct*128+c]   (ct = slab*8 + ct')
    wfc_t = np.ascontiguousarray(
        wfc_f[:nl].reshape(nl, NEO, 128, 32, 128).transpose(0, 3, 2, 1, 4).astype(bf)
    )
    # wfc2[l, slab, ch, p, k, c] = Wfc2[l, slab*1024 + k*128 + p, ch*512 + c]
    f2 = np.asarray(fc2_w, f)[:nl]
    wfc2_t = np.ascontiguousarray(
        f2.reshape(nl, 4, NEO, 128, 2, 512).transpose(0, 1, 4, 3, 2, 5).astype(bf)
    )

    # lm_head: wtet[vc, p, eo, c] = (wte*lnf)[vshard + vc*512 + c, eo*128 + p]
    wtes = wte * np.asarray(lnf_w, f)[None, :]
    shards = []
    for cshard in range(NCORES):
        sl = wtes[cshard * VSH : min(V, (cshard + 1) * VSH)]  # [vs, E]
        pad = np.zeros((VSP, E), f)
        pad[: sl.shape[0]] = sl
        sh = pad.reshape(VSP // 512, 512, NEO, 128).transpose(0, 3, 2, 1).astype(bf)
        shards.append(np.ascontiguousarray(sh))

    return np.ascontiguousarray(x0_all, f), wqk_t, wv_t, wproj_t, wfc_t, wfc2_t, shards


def kernel(idx, wte, wpe, ln1_w, ln1_b, attn_w, attn_b, proj_w, proj_b,
           ln2_w, ln2_b, fc_w, fc_b, fc2_w, fc2_b, lnf_w, lnf_b):
    import ml_dtypes
    from concourse.bass_utils import run_bass_kernel_spmd

    x0_all, wqk_t, wv_t, wproj_t, wfc_t, wfc2_t, shards = _host_prep(
        idx, wte, wpe, ln1_w, ln1_b, attn_w, attn_b, proj_w, proj_b,
        ln2_w, ln2_b, fc_w, fc_b, fc2_w, fc2_b, lnf_w, lnf_b, NL)

    if "p1" not in _CACHE:
        _CACHE["p1"] = _build_phase1(NL)
    nc1 = _CACHE["p1"]
    in_maps = [
        {"x0": x0_all[c], "wqk": wqk_t, "wv": wv_t, "wproj": wproj_t,
         "wfc": wfc_t, "wfc2": wfc2_t}
        for c in range(NCORES)
    ]
    trace = os.environ.get("GPT_TRACE", "0") == "1"
    r1 = run_bass_kernel_spmd(nc1, in_maps, core_ids=list(range(NCORES)), trace=trace)
    _CACHE["r1"] = r1
    xall = np.stack([r1.results[c]["xlast"][0] for c in range(NCORES)])  # [8, E]
    # xt[p, eo, s] = xall[s, eo*128+p]
    xt = np.ascontiguousarray(
        xall.reshape(NCORES, NEO, 128).transpose(2, 1, 0).astype(ml_dtypes.bfloat16)
    )

    if "p2" not in _CACHE:
        _CACHE["p2"] = _build_phase2()
    nc2 = _CACHE["p2"]
    in_maps2 = [{"xt": xt, "wtet": shards[c]} for c in range(NCORES)]
    r2 = run_bass_kernel_spmd(nc2, in_maps2, core_ids=list(range(NCORES)), trace=trace)
    _CACHE["r2"] = r2

    logits = np.zeros((NCORES, 1, V), np.float32)
    for c in range(NCORES):
        w = min(V, (c + 1) * VSH) - c * VSH
        logits[:, 0, c * VSH : c * VSH + w] = r2.results[c]["lg"][:, :w]
    return logits


# revision 30
# speedup vs baseline: 1.0910x; 1.0004x over previous
"""GPT-2 (L=12, E=1024, H=16, T=1024, B=8) forward on 8 Trainium2 NeuronCores.

Strategy: data-parallel over batch (1 sequence per core) for the 12 transformer
layers; vocab-parallel lm_head (each core computes logits for a V/8 shard of the
vocabulary for all 8 sequences) as a second tiny NEFF, with the 8 last-position
hidden vectors gathered on host between the phases.

v2 (all-bf16 matmul datapath, fp32 residual/PSUM):
  - weights pre-cast to bf16 and pre-tiled on host so every weight load is one
    large contiguous DMA
  - LN output transposed via DMA-transpose (xbar) instead of PE transposes
  - attention: heads processed in pairs (even head on partitions 0-63, odd on
    64-127) so the two QK matmuls row-tile the PE array concurrently
  - softmax denominator comes free from an appended ones-column in V
    (out row 64 of the AV matmul), replacing the ones-matmul accumulation
  - causal mask applied as a post-exp zeroing affine_select on GpSimd
  - layer 11 computes Q/attention/proj/MLP only for the last 128 tokens
    (only the last position's logits are needed downstream)

Host-side preprocessing (all linear folds, no model compute):
  - embedding gather x0 = wte[idx] + wpe  (pure indexing)
  - layernorm scale folded into the following matmul weights
  - 1/sqrt(D) folded into W_q
  - wte transposed (+ lnf scale) for the lm_head
"""

import os
import sys

import numpy as np

sys.path.insert(0, "/opt/trn_rl_repo")

V, BLK, L, H, E = 50257, 1024, 12, 16, 1024
T = 1024
D = E // H  # 64
NCORES = 8
E3 = 3 * E
E4 = 4 * E
NTT = T // 128  # 8 token tiles
NEO = E // 128  # 8 embed tiles
VSH = (V + NCORES - 1) // NCORES  # 6283 vocab shard
VSP = 13 * 512  # 6656 padded shard width
NL = int(os.environ.get("GPT_NL", str(L)))
TRUNC_LAST = os.environ.get("GPT_TRUNC", "1") == "1"

_CACHE = {}

# attT column offsets for the compacted causal layout, per q-window start
def _offs(qlo):
    offs, col = [], 0
    for j in range(NTT):
        offs.append(col)
        col += T - max(j * 128, qlo)
    return offs, col


def _build_phase1(nl):
    import concourse.mybir as mybir
    import concourse.tile as tile
    from concourse import bacc

    f32 = mybir.dt.float32
    bf16 = mybir.dt.bfloat16
    AF = mybir.ActivationFunctionType
    ALU = mybir.AluOpType

    nc = bacc.Bacc("TRN2", target_bir_lowering=False)

    x0 = nc.dram_tensor("x0", [T, E], f32, kind="ExternalInput")
    wqk = nc.dram_tensor("wqk", [nl, 16, 128, NEO, 128], bf16, kind="ExternalInput")
    wv = nc.dram_tensor("wv", [nl, 2, 128, NEO, 512], bf16, kind="ExternalInput")
    wproj = nc.dram_tensor("wproj", [nl, 2, 128, NEO, 512], bf16, kind="ExternalInput")
    wfc = nc.dram_tensor("wfc", [nl, 32, 128, NEO, 128], bf16, kind="ExternalInput")
    wfc2 = nc.dram_tensor("wfc2", [nl, 4, 2, 128, NEO, 512], bf16, kind="ExternalInput")
    xlast = nc.dram_tensor("xlast", [1, E], f32, kind="ExternalOutput")

    ATT_W = 4608  # full compacted attT width per head
    VGW = 16 * 65  # V block per tt: 16 heads x (64 cols + ones col)

    with tile.TileContext(nc) as tc:
        import contextlib

        ctx = contextlib.ExitStack()
        with ctx:
            singles = ctx.enter_context(tc.tile_pool(name="singles", bufs=1))
            avs = ctx.enter_context(tc.tile_pool(name="avs", bufs=2))
            wl = ctx.enter_context(tc.tile_pool(name="wl", bufs=3))  # lhsT ct tiles
            wr = ctx.enter_context(tc.tile_pool(name="wr", bufs=3))  # rhs tiles
            hpool = ctx.enter_context(tc.tile_pool(name="hpool", bufs=2))
            stat = ctx.enter_context(tc.tile_pool(name="stat", bufs=2))
            bc = ctx.enter_context(tc.tile_pool(name="bc", bufs=2))
            dram = ctx.enter_context(tc.tile_pool(name="dram", bufs=2, space="DRAM"))
            scrA = ctx.enter_context(tc.tile_pool(name="scrA", bufs=2))
            scrB = ctx.enter_context(tc.tile_pool(name="scrB", bufs=1))
            # PSUM: pmm (evacuated mm outputs) and pqk (attention scores) are
            # separate pools so Q/K/V/MLP matmuls never block on the exp chain
            pmm = ctx.enter_context(tc.tile_pool(name="pmm", bufs=2, space="PSUM"))
            pqk = ctx.enter_context(tc.tile_pool(name="pqk", bufs=2, space="PSUM"))
            pav = ctx.enter_context(tc.tile_pool(name="pav", bufs=1, space="PSUM"))

            # ---- persistent tiles ----
            X = singles.tile([128, NTT, T], f32)  # residual [tp, tt, e]
            HT = singles.tile([128, NEO, T], bf16)  # ln-out transposed [ep, eo, t]
            AOT = singles.tile([128, NEO, T], bf16)  # attn outT [cp, co, t]
            eps_t = singles.tile([128, 1], f32)
            nc.gpsimd.memset(eps_t, 1e-5)

            # ---- load x0 ----
            x0v = x0[:, :].rearrange("(tt p) e -> p tt e", p=128)
            for tt in range(NTT):
                nc.sync.dma_start(X[:, tt, :], x0v[:, tt, :])

            def layernorm(tts, l, which):
                """LN(X[:,tt,:]) -> HT[:, :, tt*128:(tt+1)*128] via DMA transpose."""
                for tt in tts:
                    st = stat.tile([128, 2, 6], f32, tag="bnst", name=f"st{l}_{which}_{tt}")
                    for c in range(2):
                        nc.vector.bn_stats(st[:, c, :], X[:, tt, c * 512 : (c + 1) * 512])
                    mv = stat.tile([128, 2], f32, tag="bnmv", name=f"mv{l}_{which}_{tt}")
                    nc.vector.bn_aggr(mv, st)
                    rstd = stat.tile([128, 1], f32, tag="rstd", name=f"rs{l}_{which}_{tt}")
                    nc.scalar.activation(rstd, mv[:, 1:2], AF.Sqrt, bias=eps_t)
                    nc.vector.reciprocal(rstd, rstd)
                    h = hpool.tile([128, T], bf16, tag="h", name=f"h{l}_{which}_{tt}")
                    nc.vector.tensor_scalar(
                        out=h,
                        in0=X[:, tt, :],
                        scalar1=mv[:, 0:1],
                        scalar2=rstd,
                        op0=ALU.subtract,
                        op1=ALU.mult,
                    )
                    nc.sync.dma_start_transpose(HT[:, :, tt * 128 : (tt + 1) * 128], h)

            def mm_lhsw(dst_fn, wdram_ct, t_lo, t_hi, act, nm):
                """dst(ch0, w) <- W_ct.T @ HT[:, :, ch0:ch0+w] per 512 chunk."""
                wt = wl.tile([128, NEO, 128], bf16, tag="wl", name=f"wt{nm}")
                nc.sync.dma_start(wt, wdram_ct)
                for ch0 in range(t_lo, t_hi, 512):
                    w = min(512, t_hi - ch0)
                    pt = pmm.tile([128, 512], f32, tag="pmm", name=f"pt{nm}_{ch0}")
                    for eo in range(NEO):
                        nc.tensor.matmul(
                            pt[:, :w],
                            wt[:, eo, :],
                            HT[:, eo, ch0 : ch0 + w],
                            start=(eo == 0),
                            stop=(eo == NEO - 1),
                        )
                    if act is None:
                        nc.vector.tensor_copy(dst_fn(ch0, w), pt[:, :w])
                    else:
                        nc.scalar.activation(dst_fn(ch0, w), pt[:, :w], act)

            # LN1 of layer 0; later layers' LN1 is emitted inside the previous
            # layer's fc2 tail so it pipelines with the remaining MLP matmuls
            layernorm(range(NTT), 0, 0)

            for l in range(nl):
                last = TRUNC_LAST and (l == nl - 1) and (nl == L)
                qlo = T - 128 if last else 0
                tts = [NTT - 1] if last else list(range(NTT))
                offs, attw = _offs(qlo)

                # ===== attention =====
                VA = scrB.tile([128, NTT * VGW + 2 * ATT_W], bf16, tag="scrB", name=f"va{l}")
                VG = VA[:, : NTT * VGW].rearrange("p (tt h c) -> p tt h c", tt=NTT, h=16)

                def attT(slot, col, n):  # slot = head_in_pair (one pair in flight)
                    return VA[:, NTT * VGW + slot * ATT_W + col : NTT * VGW + slot * ATT_W + col + n]

                # ones columns for the free softmax denominator
                nc.gpsimd.memset(VG[:, :, :, 64:65], 1.0)

                # V for both groups first (so AV can chase QK/exp per pair)
                for g in range(2):
                    wvt = wr.tile([128, NEO, 512], bf16, tag="wr", name=f"wv{l}_{g}")
                    nc.sync.dma_start(wvt, wv[l, g])
                    for tt in range(NTT):
                        pv = pmm.tile([128, 512], f32, tag="pmm", name=f"pv{l}_{g}_{tt}")
                        for eo in range(NEO):
                            nc.tensor.matmul(
                                pv,
                                HT[:, eo, tt * 128 : (tt + 1) * 128],
                                wvt[:, eo, :],
                                start=(eo == 0),
                                stop=(eo == NEO - 1),
                            )
                        nc.scalar.activation(
                            VG[:, tt, g * 8 : (g + 1) * 8, 0:64],
                            pv.rearrange("p (h c) -> p h c", h=8),
                            AF.Copy,
                        )

                QT = scrA.tile([128, NEO, T], bf16, tag="scrA", name=f"qt{l}")
                KT = scrA.tile([128, NEO, T], bf16, tag="scrA", name=f"kt{l}")

                for p in range(NEO):  # head pair p = heads (2p, 2p+1)
                    mm_lhsw(lambda c0, w, p=p: QT[:, p, c0 : c0 + w], wqk[l, p], qlo, T, None, f"q{l}_{p}")
                    mm_lhsw(lambda c0, w, p=p: KT[:, p, c0 : c0 + w], wqk[l, 8 + p], 0, T, None, f"k{l}_{p}")

                    # --- QK for the pair (row-tiled: even rows 0-63, odd 64-127) ---
                    # consecutive k-blocks share a psum tile (and one exp) while
                    # their combined q-width fits the 2-bank tile
                    jgroups, cur, curw = [], [], 0
                    for j in range(NTT):
                        qn = T - max(j * 128, qlo)
                        if curw + qn > 1024:
                            jgroups.append(cur)
                            cur, curw = [], 0
                        cur.append((j, curw, qn))
                        curw += qn
                    jgroups.append(cur)
                    for gi, grp in enumerate(jgroups):
                        gw = sum(qn for _, _, qn in grp)
                        pe_t = pqk.tile([128, T], f32, tag="pqk", name=f"pe{l}_{p}_{gi}")
                        po_t = pqk.tile([128, T], f32, tag="pqk", name=f"po{l}_{p}_{gi}")
                        for j, lo, qn in grp:
                            qs = max(j * 128, qlo)
                            for ch0 in range(0, qn, 512):
                                w = min(512, qn - ch0)
                                nc.tensor.matmul(
                                    pe_t[:, lo + ch0 : lo + ch0 + w],
                                    KT[0:64, p, j * 128 : (j + 1) * 128],
                                    QT[0:64, p, qs + ch0 : qs + ch0 + w],
                                    start=True,
                                    stop=True,
                                )
                                nc.tensor.matmul(
                                    po_t[:, lo + ch0 : lo + ch0 + w],
                                    KT[64:128, p, j * 128 : (j + 1) * 128],
                                    QT[64:128, p, qs + ch0 : qs + ch0 + w],
                                    start=True,
                                    stop=True,
                                )
                        o0 = offs[grp[0][0]]
                        nc.scalar.activation(attT(0, o0, gw), pe_t[:, :gw], AF.Exp)
                        nc.scalar.activation(attT(1, o0, gw), po_t[:, :gw], AF.Exp)
                        for j, lo, qn in grp:
                            if j * 128 >= qlo:  # diagonal: zero strict upper triangle
                                for hh in range(2):
                                    nc.gpsimd.affine_select(
                                        out=attT(hh, offs[j], 128),
                                        in_=attT(hh, offs[j], 128),
                                        compare_op=ALU.is_ge,
                                        fill=0.0,
                                        base=0,
                                        pattern=[[1, 128]],
                                        channel_multiplier=-1,
                                    )

                    # --- AV + free denominator for both heads of the pair ---
                    for hh in range(2):
                        h_glob = 2 * p + hh
                        wd = T - qlo
                        avp = pav.tile([65, 1024], f32, tag="pav", name=f"av{l}_{h_glob}")
                        for ca0 in range(qlo, T, 512):
                            cw = min(512, T - ca0)
                            js = [j for j in range(NTT) if j * 128 < ca0 + cw]
                            for ji, j in enumerate(js):
                                s = max(ca0, j * 128)
                                w = ca0 + cw - s
                                nc.tensor.matmul(
                                    avp[:, s - qlo : s - qlo + w],
                                    VG[:, j, h_glob, :],
                                    attT(hh, offs[j] + s - max(j * 128, qlo), w),
                                    start=(ji == 0),
                                    stop=(ji == len(js) - 1),
                                    skip_group_check=True,
                                )
                        # one copy evacuates value rows + denominator row,
                        # freeing the PSUM accumulator for the next head fast
                        av_s = avs.tile([65, 1024], f32, tag="avs", name=f"avs{l}_{h_glob}")
                        nc.vector.tensor_copy(av_s[:, :wd], avp[:, :wd])
                        # full-tile AP (partition 0, 65 rows): reciprocal_approx_fast
                        # rejects single-partition slices at nonzero base partition
                        rden = stat.tile([65, 1024], f32, tag="rden", name=f"rd{l}_{h_glob}")
                        nc.vector.reciprocal_approx_fast(out=rden[:, :wd], in_=av_s[:, :wd])
                        # den round-trip on the SWDGE ring: these DMAs wait on
                        # compute, and on the sync ring they head-of-line block
                        # the (dep-free) weight loads queued behind them
                        rdd = dram.tile([1, 1024], f32, tag="rdd", name=f"rdd{l}_{h_glob}")
                        nc.gpsimd.dma_start(rdd[:, :wd], rden[64:65, :wd])
                        rdb = bc.tile([64, 1024], f32, tag="rdb", name=f"rdb{l}_{h_glob}")
                        nc.gpsimd.dma_start(rdb[:, :wd], rdd[:, :wd].to_broadcast([64, wd]))
                        co, ro2 = h_glob // 2, (h_glob % 2) * 64
                        nc.vector.tensor_tensor(
                            AOT[ro2 : ro2 + 64, co, qlo:T], av_s[0:64, :wd], rdb[:, :wd], ALU.mult
                        )

                # ===== proj + residual =====
                wp0 = wr.tile([128, NEO, 512], bf16, tag="wr", name=f"wp0{l}")
                wp1 = wr.tile([128, NEO, 512], bf16, tag="wr", name=f"wp1{l}")
                nc.sync.dma_start(wp0, wproj[l, 0])
                nc.sync.dma_start(wp1, wproj[l, 1])
                for tt in tts:
                    for chi, wpt in ((0, wp0), (1, wp1)):
                        pp = pmm.tile([128, 512], f32, tag="pmm", name=f"pp{l}_{tt}_{chi}")
                        for k in range(NEO):
                            nc.tensor.matmul(
                                pp,
                                AOT[:, k, tt * 128 : (tt + 1) * 128],
                                wpt[:, k, :],
                                start=(k == 0),
                                stop=(k == NEO - 1),
                            )
                        xs = X[:, tt, chi * 512 : chi * 512 + 512]
                        nc.vector.tensor_tensor(xs, xs, pp, ALU.add)
                    # LN2 for this token tile rides the proj loop so fc1 never
                    # waits on a serialized LN chain after proj completes
                    layernorm([tt], l, 1)

                # ===== mlp =====  (LN2 already emitted inside the proj loop)
                FC2A = scrB.tile([128, NTT, T], f32, tag="scrB", name=f"fc2a{l}")
                for slab in range(4):
                    H1T = scrA.tile([128, NEO, T], bf16, tag="scrA", name=f"h1t{l}_{slab}")
                    for ct in range(NEO):
                        mm_lhsw(
                            lambda c0, w, ct=ct, H1T=H1T: H1T[:, ct, c0 : c0 + w],
                            wfc[l, slab * 8 + ct],
                            qlo,
                            T,
                            AF.Gelu_apprx_tanh,
                            f"f{l}_{slab}_{ct}",
                        )
                    w20 = wr.tile([128, NEO, 512], bf16, tag="wr", name=f"w20{l}_{slab}")
                    w21 = wr.tile([128, NEO, 512], bf16, tag="wr", name=f"w21{l}_{slab}")
                    nc.sync.dma_start(w20, wfc2[l, slab, 0])
                    nc.sync.dma_start(w21, wfc2[l, slab, 1])
                    for tt in tts:
                        for chi, w2t in ((0, w20), (1, w21)):
                            p2 = pmm.tile([128, 512], f32, tag="pmm", name=f"p2{l}_{slab}_{tt}_{chi}")
                            for k in range(NEO):
                                nc.tensor.matmul(
                                    p2,
                                    H1T[:, k, tt * 128 : (tt + 1) * 128],
                                    w2t[:, k, :],
                                    start=(k == 0),
                                    stop=(k == NEO - 1),
                                )
                            sl = slice(chi * 512, chi * 512 + 512)
                            if slab == 0:
                                nc.vector.tensor_tensor(FC2A[:, tt, sl], X[:, tt, sl], p2, ALU.add)
                            elif slab < 3:
                                nc.vector.tensor_tensor(FC2A[:, tt, sl], FC2A[:, tt, sl], p2, ALU.add)
                            else:
                                nc.vector.tensor_tensor(X[:, tt, sl], FC2A[:, tt, sl], p2, ALU.add)
                        if slab == 3 and l + 1 < nl:
                            # next layer's LN1 for this token tile rides the
                            # fc2 tail so the layer boundary never idles PE
                            layernorm([tt], l + 1, 0)

            # ===== final layernorm on last token tile, emit last row =====
            st = stat.tile([128, 2, 6], f32, tag="bnst", name="stf")
            for c in range(2):
                nc.vector.bn_stats(st[:, c, :], X[:, NTT - 1, c * 512 : (c + 1) * 512])
            mv = stat.tile([128, 2], f32, tag="bnmv", name="mvf")
            nc.vector.bn_aggr(mv, st)
            rstd = stat.tile([128, 1], f32, tag="rstd", name="rsf")
            nc.scalar.activation(rstd, mv[:, 1:2], AF.Sqrt, bias=eps_t)
            nc.vector.reciprocal(rstd, rstd)
            xn = hpool.tile([128, T], f32, tag="xn", name="xnf", bufs=1)
            nc.vector.tensor_scalar(
                out=xn,
                in0=X[:, NTT - 1, :],
                scalar1=mv[:, 0:1],
                scalar2=rstd,
                op0=ALU.subtract,
                op1=ALU.mult,
            )
            nc.sync.dma_start(xlast[:, :], xn[127:128, :])

    nc.compile()
    return nc


def _build_phase2():
    import concourse.mybir as mybir
    import concourse.tile as tile
    from concourse import bacc

    f32 = mybir.dt.float32
    bf16 = mybir.dt.bfloat16
    AF = mybir.ActivationFunctionType

    nc = bacc.Bacc("TRN2", target_bir_lowering=False)
    xt_d = nc.dram_tensor("xt", [128, NEO, NCORES], bf16, kind="ExternalInput")
    wtet = nc.dram_tensor("wtet", [VSP // 512, 128, NEO, 512], bf16, kind="ExternalInput")
    lg = nc.dram_tensor("lg", [NCORES, VSP], f32, kind="ExternalOutput")

    with tile.TileContext(nc) as tc:
        with (
            tc.tile_pool(name="s", bufs=1) as s,
            tc.tile_pool(name="w", bufs=6) as w,
            tc.tile_pool(name="o", bufs=4) as o,
            tc.tile_pool(name="p", bufs=4, space="PSUM") as p,
        ):
            xt = s.tile([128, NEO, NCORES], bf16)
            nc.sync.dma_start(xt, xt_d[:, :, :])
            for vc in range(VSP // 512):
                wt = w.tile([128, NEO, 512], bf16, tag="w", name=f"w{vc}")
                eng = nc.sync if vc % 2 == 0 else nc.scalar
                eng.dma_start(wt, wtet[vc])
                pt = p.tile([NCORES, 512], f32, tag="p", name=f"p{vc}")
                for eo in range(NEO):
                    nc.tensor.matmul(pt, xt[:, eo, :], wt[:, eo, :], start=(eo == 0), stop=(eo == NEO - 1))
                ot = o.tile([NCORES, 512], f32, tag="o", name=f"o{vc}")
                nc.scalar.activation(ot, pt, AF.Copy)
                nc.sync.dma_start(lg[:, vc * 512 : (vc + 1) * 512], ot)
    nc.compile()
    return nc


def _host_prep(idx, wte, wpe, ln1_w, ln1_b, attn_w, attn_b, proj_w, proj_b,
               ln2_w, ln2_b, fc_w, fc_b, fc2_w, fc2_b, lnf_w, lnf_b, nl):
    import ml_dtypes

    f = np.float32
    bf = ml_dtypes.bfloat16
    idx = np.asarray(idx)
    wte = np.asarray(wte, f)
    wpe = np.asarray(wpe, f)
    x0_all = wte[idx] + wpe[None, :T]  # [8, T, E]

    attn_w = np.asarray(attn_w, f)
    ln1_w = np.asarray(ln1_w, f)
    fc_w = np.asarray(fc_w, f)
    ln2_w = np.asarray(ln2_w, f)

    # fold ln scale into following weights; fold 1/sqrt(D) into W_q
    wqkv = attn_w * ln1_w[:, :, None]
    wqkv[:, :, :E] *= 1.0 / np.sqrt(D)
    wfc_f = fc_w * ln2_w[:, :, None]

    # biases: must be zero (true for this model)
    bqkv = np.einsum("le,lec->lc", np.asarray(ln1_b, f), attn_w) + np.asarray(attn_b, f)
    bfc = np.einsum("le,lec->lc", np.asarray(ln2_b, f), fc_w) + np.asarray(fc_b, f)
    for nm, b in [("bqkv", bqkv), ("proj_b", np.asarray(proj_b, f)),
                  ("bfc", bfc), ("fc2_b", np.asarray(fc2_b, f)),
                  ("lnf_b", np.asarray(lnf_b, f))]:
        assert np.abs(b).max() == 0.0, f"nonzero bias {nm} not supported by this kernel"

    # --- pre-tiled bf16 weight layouts (one contiguous DMA per tile load) ---
    # wqk[l, ct, p, eo, c] = Wqkv[l, eo*128+p, ct*128+c] (ct 0-7 Q, 8-15 K)
    qk = wqkv[:nl, :, : 2 * E]  # [nl, E, 2E]
    wqk_t = np.ascontiguousarray(
        qk.reshape(nl, NEO, 128, 16, 128).transpose(0, 3, 2, 1, 4).astype(bf)
    )
    # wv[l, g, p, eo, c] = Wqkv[l, eo*128+p, 2E + g*512 + c]
    vv = wqkv[:nl, :, 2 * E :]  # [nl, E, E]
    wv_t = np.ascontiguousarray(
        vv.reshape(nl, NEO, 128, 2, 512).transpose(0, 3, 2, 1, 4).astype(bf)
    )
    # wproj[l, ch, p, k, c] = Wproj[l, k*128+p, ch*512+c]
    pr = np.asarray(proj_w, f)[:nl]
    wproj_t = np.ascontiguousarray(
        pr.reshape(nl, NEO, 128, 2, 512).transpose(0, 3, 2, 1, 4).astype(bf)
    )
    # wfc[l, ct, p, eo, c] = Wfc[l, eo*128+p, ct*128+c]   (ct = slab*8 + ct')
    wfc_t = np.ascontiguousarray(
        wfc_f[:nl].reshape(nl, NEO, 128, 32, 128).transpose(0, 3, 2, 1, 4).astype(bf)
    )
    # wfc2[l, slab, ch, p, k, c] = Wfc2[l, slab*1024 + k*128 + p, ch*512 + c]
    f2 = np.asarray(fc2_w, f)[:nl]
    wfc2_t = np.ascontiguousarray(
        f2.reshape(nl, 4, NEO, 128, 2, 512).transpose(0, 1, 4, 3, 2, 5).astype(bf)
    )

    # lm_head: wtet[vc, p, eo, c] = (wte*lnf)[vshard + vc*512 + c, eo*128 + p]
    wtes = wte * np.asarray(lnf_w, f)[None, :]
    shards = []
    for cshard in range(NCORES):
        sl = wtes[cshard * VSH : min(V, (cshard + 1) * VSH)]  # [vs, E]
        pad = np.zeros((VSP, E), f)
        pad[: sl.shape[0]] = sl
        sh = pad.reshape(VSP // 512, 512, NEO, 128).transpose(0, 3, 2, 1).astype(bf)
        shards.append(np.ascontiguousarray(sh))

    return np.ascontiguousarray(x0_all, f), wqk_t, wv_t, wproj_t, wfc_t, wfc2_t, shards


def kernel(idx, wte, wpe, ln1_w, ln1_b, attn_w, attn_b, proj_w, proj_b,
           ln2_w, ln2_b, fc_w, fc_b, fc2_w, fc2_b, lnf_w, lnf_b):
    import ml_dtypes
    from concourse.bass_utils import run_bass_kernel_spmd

    x0_all, wqk_t, wv_t, wproj_t, wfc_t, wfc2_t, shards = _host_prep(
        idx, wte, wpe, ln1_w, ln1_b, attn_w, attn_b, proj_w, proj_b,
        ln2_w, ln2_b, fc_w, fc_b, fc2_w, fc2_b, lnf_w, lnf_b, NL)

    if "p1" not in _CACHE:
        _CACHE["p1"] = _build_phase1(NL)
    nc1 = _CACHE["p1"]
    in_maps = [
        {"x0": x0_all[c], "wqk": wqk_t, "wv": wv_t, "wproj": wproj_t,
         "wfc": wfc_t, "wfc2": wfc2_t}
        for c in range(NCORES)
    ]
    trace = os.environ.get("GPT_TRACE", "0") == "1"
    r1 = run_bass_kernel_spmd(nc1, in_maps, core_ids=list(range(NCORES)), trace=trace)
    _CACHE["r1"] = r1
    xall = np.stack([r1.results[c]["xlast"][0] for c in range(NCORES)])  # [8, E]
    # xt[p, eo, s] = xall[s, eo*128+p]
    xt = np.ascontiguousarray(
        xall.reshape(NCORES, NEO, 128).transpose(2, 1, 0).astype(ml_dtypes.bfloat16)
    )

    if "p2" not in _CACHE:
        _CACHE["p2"] = _build_phase2()
    nc2 = _CACHE["p2"]
    in_maps2 = [{"xt": xt, "wtet": shards[c]} for c in range(NCORES)]
    r2 = run_bass_kernel_spmd(nc2, in_maps2, core_ids=list(range(NCORES)), trace=trace)
    _CACHE["r2"] = r2

    logits = np.zeros((NCORES, 1, V), np.float32)
    for c in range(NCORES):
        w = min(V, (c + 1) * VSH) - c * VSH
        logits[:, 0, c * VSH : c * VSH + w] = r2.results[c]["lg"][:, :w]
    return logits
